# revision 1
# baseline (speedup 1.0000x reference)
"""Trainium2 Bass kernel for nn_ClockAwareGNN (segment_reduce).

Model (reference, fp32):
    gp   = segment_mean(x, batch) @ W_base + b_base            # [B, 1]
    h    = relu(clock @ W1 + b1) @ W2 + b2                     # [N, 16]
    cp   = segment_mean(h, batch)                              # [B, 16]
    out  = relu([gp | cp] @ W3 + b3) @ W4 + b4                 # [B, 1]

Everything after the segment reductions is affine in per-graph quantities, so
the heavy per-node work collapses to three fused segment reductions:
    Sx[g]  = sum of x rows in graph g          (128 cols)
    Sr[g]  = sum of r rows in graph g          (R cols)
    cnt[g] = node count of graph g
where r is either the raw clock (R=1; exact when b1 == 0 and clock >= 0 since
relu(c*W1) == c*relu(W1) elementwise for c >= 0) or the host-computed
relu(clock @ W1 + b1) (R=16 fallback).

Device strategy (per core, 8-way data-parallel by graph):
  - nodes arrive as 128-row tiles; batch ids are sorted so each tile touches
    <= 2 graphs inside one 32-graph "window".
  - DVE builds one-hot assign tiles [128 nodes, 32 graphs] for a whole
    super-tile in one is_equal op (broadcast AP vs an iota pattern).
  - PE accumulates assign.T @ payload into PSUM [128 graphs, C1+129] fp32.
    x is shipped as bf16 hi (+ r hi/lo) in one matmul and a 512-scaled
    fp8e4m3 lo-correction (+ a ones column providing counts) in a second
    matmul — 3 B/element of x traffic with ~2^-13 effective precision.
  - tiny vector-engine epilogue computes the folded per-graph MLP.
"""

import math
import sys
import types

import numpy as np
import ml_dtypes

import concourse.bass as bass
import concourse.bacc as bacc
import concourse.tile as tile
from concourse import mybir
from concourse.bass_utils import run_bass_kernel_spmd


def _ensure_axon_hooks():
    """bass_utils' trace path does `from antenv.axon_hooks import ...`;
    some agent images lack that submodule. Install it (with the real NTFF
    hook when available) so trace=True degrades gracefully instead of
    raising ModuleNotFoundError."""
    try:
        import antenv  # noqa: F401
        import antenv.axon_hooks  # noqa: F401
        return
    except ImportError:
        pass
    try:
        import antenv
    except ImportError:
        return
    mod = types.ModuleType("antenv.axon_hooks")
    state = {"hook": None}
    mod.set_axon_ntff_profile_hook = lambda h: state.__setitem__("hook", h)
    mod.get_axon_ntff_profile_hook = lambda: state["hook"]
    sys.modules["antenv.axon_hooks"] = mod
    antenv.axon_hooks = mod
    try:
        from trn_agent_boot.trn_boot import _ntff_profile_via_ctypes
        mod.set_axon_ntff_profile_hook(
            _ntff_profile_via_ctypes("/opt/axon/libaxon_pjrt.so"))
    except Exception:
        pass
    # the trace path also uploads the NEFF dir to a bucket; in zero-egress
    # containers that raises — fall back to the local path.
    try:
        import concourse.bass_utils as _bu
        _orig_upload = _bu.upload_artifacts

        def _safe_upload(tmpdir):
            try:
                return _orig_upload(tmpdir)
            except Exception:
                return str(tmpdir)

        _bu.upload_artifacts = _safe_upload
    except Exception:
        pass


_ensure_axon_hooks()

BF16 = ml_dtypes.bfloat16
F8 = ml_dtypes.float8_e4m3

N_CORES = 8
N_GRAPHS = 1024
D = 128                 # feature dim of x
GPC = N_GRAPHS // N_CORES   # graphs per core = 128
W = 32                  # one-hot window width (PSUM partition alignment unit)
WPC = GPC // W          # windows per core = 4
ST = 32                 # node-tiles per DMA super-tile
LO_SCALE = 512.0        # fp8 lo-correction pre-scale (2^9)


def _build_program(S, C1, R, mixed_lhsT):
    """Build the SPMD Bass/Tile program. Shapes are static; per-core data
    differences live entirely in the input tensors.

    S:  number of super-tiles (each ST node-tiles of 128 nodes)
    C1: bf16 payload column count = 128 + 2*R
    mixed_lhsT: if True, the fp8 lo matmul reuses the bf16 assign tile
    """
    fp32 = mybir.dt.float32
    bf16 = mybir.dt.bfloat16
    f8 = mybir.dt.float8e4
    n_tiles = S * ST
    T_w = n_tiles // WPC
    DL = D + 1             # fp8 lo block width: lo cols + count-of-ones col
    C_ps = C1 + DL         # psum cols: bf16 block + lo block

    nc = bacc.Bacc("TRN2", target_bir_lowering=False, debug=False,
                   num_devices=N_CORES)

    xcc = nc.dram_tensor("xcc", [S, 128, ST * C1], bf16, kind="ExternalInput").ap()
    xlo = nc.dram_tensor("xlo", [S, 128, ST * DL], f8, kind="ExternalInput").ap()
    brs = nc.dram_tensor("brs", [128, S * ST], bf16, kind="ExternalInput").ap()
    iota_c = nc.dram_tensor("iota_c", [128, ST * W], bf16, kind="ExternalInput").ap()
    wbase_b = nc.dram_tensor("wbase_b", [128, D], fp32, kind="ExternalInput").ap()
    v1_b = nc.dram_tensor("v1_b", [128, 32], fp32, kind="ExternalInput").ap()
    m2_b = nc.dram_tensor("m2_b", [128, R * 32], fp32, kind="ExternalInput").ap()
    v0_b = nc.dram_tensor("v0_b", [128, 32], fp32, kind="ExternalInput").ap()
    w4_b = nc.dram_tensor("w4_b", [128, 32], fp32, kind="ExternalInput").ap()
    bb_t = nc.dram_tensor("bb_t", [128, 1], fp32, kind="ExternalInput").ap()
    b4_t = nc.dram_tensor("b4_t", [128, 1], fp32, kind="ExternalInput").ap()
    out_d = nc.dram_tensor("out", [128, 1], fp32, kind="ExternalOutput").ap()

    with tile.TileContext(nc) as tc:
        with (
            tc.tile_pool(name="consts", bufs=1) as cpool,
            tc.tile_pool(name="xin", bufs=8) as xpool,
            tc.tile_pool(name="loin", bufs=8) as lpool,
            tc.tile_pool(name="assign", bufs=3) as apool,
            tc.tile_pool(name="epi", bufs=1) as epool,
            tc.tile_pool(name="ps", bufs=1, space="PSUM") as ppool,
        ):
            # ---- constants ----
            iota_t = cpool.tile([128, ST * W], bf16, tag="iota")
            nc.sync.dma_start(iota_t[:], iota_c)
            # whole-run batch-rel ids: one small DMA instead of one per super
            brall = cpool.tile([128, S * ST], bf16, tag="brall")
            nc.sync.dma_start(brall[:], brs)
            wb_t = cpool.tile([128, D], fp32, tag="wb")
            nc.sync.dma_start(wb_t[:], wbase_b)
            v1_t = cpool.tile([128, 32], fp32, tag="v1")
            nc.sync.dma_start(v1_t[:], v1_b)
            m2_t = cpool.tile([128, R * 32], fp32, tag="m2")
            nc.sync.dma_start(m2_t[:], m2_b)
            v0_t = cpool.tile([128, 32], fp32, tag="v0")
            nc.sync.dma_start(v0_t[:], v0_b)
            w4_t = cpool.tile([128, 32], fp32, tag="w4")
            nc.sync.dma_start(w4_t[:], w4_b)
            bbt = cpool.tile([128, 1], fp32, tag="bb")
            nc.sync.dma_start(bbt[:], bb_t)
            b4t = cpool.tile([128, 1], fp32, tag="b4")
            nc.sync.dma_start(b4t[:], b4_t)

            psum = ppool.tile([128, C_ps], fp32, tag="acc")

            # init matmul: zero weights x zero rhs, start=True claims the
            # whole bank's has_written bits so all later matmuls (start=False)
            # overwrite-on-first-touch / accumulate-after, independent of
            # window interleaving.
            zw = cpool.tile([128, 128], bf16, tag="zw")
            nc.vector.memset(zw[:], 0.0)
            zr = cpool.tile([128, C_ps], bf16, tag="zr")
            nc.vector.memset(zr[:], 0.0)
            nc.tensor.matmul(psum[:, :], zw[:], zr[:], start=True, stop=False)

            # ---- main loop ----
            for s in range(S):
                xt = xpool.tile([128, ST * C1], bf16, tag="xt")
                nc.sync.dma_start(xt[:], xcc[s])
                # second HWDGE ring (ACT) for the fp8 lo + batch-rel streams
                lt = lpool.tile([128, ST * DL], f8, tag="lt")
                nc.scalar.dma_start(lt[:], xlo[s])
                # one-hot assign for all ST tiles in one DVE op:
                # asg[p, t, j] = (iota[j] == br[p, s*ST + t])
                asg = apool.tile([128, ST * W], bf16, tag="asg")
                nc.vector.tensor_tensor(
                    asg[:].rearrange("p (t j) -> p t j", j=W),
                    iota_t[:].rearrange("p (t j) -> p t j", j=W),
                    brall[:, s * ST : (s + 1) * ST]
                        .rearrange("p (t o) -> p t o", o=1)
                        .to_broadcast((128, ST, W)),
                    op=mybir.AluOpType.is_equal,
                )
                if mixed_lhsT:
                    asg8 = asg
                else:
                    asg8 = apool.tile([128, ST * W], f8, tag="asg8")
                    nc.vector.tensor_copy(asg8[:], asg[:])
                for t in range(ST):
                    i = s * ST + t
                    w = i // T_w
                    last = i == n_tiles - 1
                    nc.tensor.matmul(
                        psum[w * W : (w + 1) * W, 0:C1],
                        asg[:, t * W : (t + 1) * W],
                        xt[:, t * C1 : (t + 1) * C1],
                        start=False,
                        stop=False,
                        tile_position=(0, w * W),
                    )
                    nc.tensor.matmul(
                        psum[w * W : (w + 1) * W, C1 : C1 + DL],
                        asg8[:, t * W : (t + 1) * W],
                        lt[:, t * DL : (t + 1) * DL],
                        start=False,
                        stop=last,
                        tile_position=(0, w * W),
                    )

            # ---- epilogue (per-graph folded MLP) ----
            sb = epool.tile([128, C_ps], fp32, tag="sb")
            nc.vector.tensor_copy(sb[:], psum[:])

            # Sx = hi_sums + lo_sums / LO_SCALE
            slo = epool.tile([128, D], fp32, tag="slo")
            nc.vector.tensor_scalar_mul(slo[:], sb[:, C1 : C1 + D], 1.0 / LO_SCALE)
            sx = epool.tile([128, D], fp32, tag="sx")
            nc.vector.tensor_add(sx[:], sb[:, 0:D], slo[:])

            sr = epool.tile([128, R], fp32, tag="sr")
            nc.vector.tensor_add(
                sr[:], sb[:, D : D + R], sb[:, D + R : D + 2 * R]
            )
            cntc = epool.tile([128, 1], fp32, tag="cnt")
            nc.vector.tensor_scalar_max(cntc[:], sb[:, C1 + D : C1 + D + 1], 1.0)
            rec = epool.tile([128, 1], fp32, tag="rec")
            nc.vector.reciprocal(rec[:], cntc[:])

            mx = epool.tile([128, D], fp32, tag="mx")
            nc.vector.tensor_scalar_mul(mx[:], sx[:], rec[:])
            mr = epool.tile([128, R], fp32, tag="mr")
            nc.vector.tensor_scalar_mul(mr[:], sr[:], rec[:])

            # gp = rowsum(mean_x * W_base) + b_base
            t1 = epool.tile([128, D], fp32, tag="t1")
            nc.vector.tensor_mul(t1[:], mx[:], wb_t[:])
            gp = epool.tile([128, 1], fp32, tag="gp")
            nc.vector.tensor_reduce(gp[:], t1[:], axis=mybir.AxisListType.X,
                                    op=mybir.AluOpType.add)
            nc.vector.tensor_add(gp[:], gp[:], bbt[:])

            # pre = gp*v1 + sum_j mr[:,j]*M2[j] + v0
            pre = epool.tile([128, 32], fp32, tag="pre")
            nc.vector.tensor_scalar_mul(pre[:], v1_t[:], gp[:])
            tmp = epool.tile([128, 32], fp32, tag="tmp")
            for j in range(R):
                nc.vector.tensor_scalar(
                    tmp[:], m2_t[:, j * 32 : (j + 1) * 32], mr[:, j : j + 1], None,
                    op0=mybir.AluOpType.mult,
                )
                nc.vector.tensor_add(pre[:], pre[:], tmp[:])
            nc.vector.tensor_add(pre[:], pre[:], v0_t[:])

            act = epool.tile([128, 32], fp32, tag="act")
            nc.scalar.activation(act[:], pre[:], mybir.ActivationFunctionType.Relu)

            # out = rowsum(act * W4) + b4
            nc.vector.tensor_mul(act[:], act[:], w4_t[:])
            oo = epool.tile([128, 1], fp32, tag="oo")
            nc.vector.tensor_reduce(oo[:], act[:], axis=mybir.AxisListType.X,
                                    op=mybir.AluOpType.add)
            nc.vector.tensor_add(oo[:], oo[:], b4t[:])

            nc.sync.dma_start(out_d, oo[:])

    nc.compile()
    return nc


def kernel(x, clock_period, batch, W_base, b_base, W1, b1, W2, b2, W3, b3, W4, b4,
           _profile=None, _mixed_lhsT=True):
    x = np.asarray(x, np.float32)
    clock = np.asarray(clock_period, np.float32).reshape(-1)
    batch = np.asarray(batch, np.int32)
    W_base = np.asarray(W_base, np.float32)
    W1 = np.asarray(W1, np.float32); b1 = np.asarray(b1, np.float32)
    W2 = np.asarray(W2, np.float32); b2 = np.asarray(b2, np.float32)
    W3 = np.asarray(W3, np.float32); b3 = np.asarray(b3, np.float32)
    W4 = np.asarray(W4, np.float32); b4 = np.asarray(b4, np.float32)
    hid = W1.shape[1]

    # r-path: exact algebraic fold when relu(c*W1 + b1) == c * relu(W1)
    fold = bool(np.all(b1 == 0.0)) and bool(clock.min() >= 0.0)
    if fold:
        R = 1
        r32 = clock[:, None]                                   # [N, 1]
        q = np.maximum(W1, 0.0) @ W2                           # [1, hid]
        M2 = q @ W3[1:, :]                                     # [1, 32]
        v0 = b2 @ W3[1:, :] + b3                               # [32]
    else:
        R = hid
        r32 = np.maximum(clock[:, None] @ W1 + b1, 0.0)        # [N, hid]
        M2 = W2 @ W3[1:, :]                                    # [hid, 32]
        v0 = b2 @ W3[1:, :] + b3

    C1 = D + 2 * R          # [xhi | rhi | rlo]; count rides in the fp8 block
    assert C1 % 2 == 0
    DL = D + 1

    # ---- shard by graph; window padding so tile->window map is static ----
    cut = np.searchsorted(batch, np.arange(0, N_GRAPHS + 1, W))
    win_nodes = np.diff(cut)
    T_w = int(math.ceil(win_nodes.max() / 128.0))
    while (WPC * T_w) % ST:
        T_w += 1
    n_tiles = WPC * T_w
    S = n_tiles // ST
    Npad = n_tiles * 128

    xhi = x.astype(BF16)
    xlo8 = ((x - xhi.astype(np.float32)) * LO_SCALE).astype(F8)
    rhi = r32.astype(BF16)
    rlo = (r32 - rhi.astype(np.float32)).astype(BF16)

    in_maps = []
    # shared constant tiles
    iota_c = np.broadcast_to(
        np.tile(np.arange(W, dtype=BF16), ST)[None, :], (128, ST * W)
    ).copy()
    wbase_b = np.broadcast_to(W_base[:, 0][None, :], (128, D)).astype(np.float32).copy()
    v1_b = np.broadcast_to(W3[0, :][None, :], (128, 32)).astype(np.float32).copy()
    m2_b = np.broadcast_to(M2.reshape(-1)[None, :], (128, R * 32)).astype(np.float32).copy()
    v0_b = np.broadcast_to(v0[None, :], (128, 32)).astype(np.float32).copy()
    w4_b = np.broadcast_to(W4[:, 0][None, :], (128, 32)).astype(np.float32).copy()
    bb_t = np.full((128, 1), float(b_base.reshape(-1)[0]), np.float32)
    b4_t = np.full((128, 1), float(b4.reshape(-1)[0]), np.float32)

    for k in range(N_CORES):
        xcc = np.zeros((Npad, C1), BF16)
        xl = np.zeros((Npad, DL), F8)
        br = np.full(Npad, -1.0, BF16)
        for wi in range(WPC):
            gw = k * WPC + wi          # global window index
            s0, e0 = int(cut[gw]), int(cut[gw + 1])
            n = e0 - s0
            o = wi * T_w * 128
            xcc[o : o + n, 0:D] = xhi[s0:e0]
            xcc[o : o + n, D : D + R] = rhi[s0:e0]
            xcc[o : o + n, D + R : D + 2 * R] = rlo[s0:e0]
            xl[o : o + n, 0:D] = xlo8[s0:e0]
            xl[o : o + n, D] = F8(1.0)
            br[o : o + n] = (batch[s0:e0] - gw * W).astype(BF16)
        brs = np.ascontiguousarray(br.reshape(S * ST, 128).T)
        # permute so each SBUF partition line is contiguous in DRAM
        xcc_p = np.ascontiguousarray(
            xcc.reshape(S, ST, 128, C1).transpose(0, 2, 1, 3)
        ).reshape(S, 128, ST * C1)
        xlo_p = np.ascontiguousarray(
            xl.reshape(S, ST, 128, DL).transpose(0, 2, 1, 3)
        ).reshape(S, 128, ST * DL)
        in_maps.append(dict(
            xcc=xcc_p, xlo=xlo_p, brs=brs, iota_c=iota_c,
            wbase_b=wbase_b, v1_b=v1_b, m2_b=m2_b, v0_b=v0_b, w4_b=w4_b,
            bb_t=bb_t, b4_t=b4_t,
        ))

    nc = _build_program(S, C1, R, _mixed_lhsT)

    kw = {}
    if _profile is not None:
        kw = dict(trace=True, **_profile)
    res = run_bass_kernel_spmd(nc, in_maps, list(range(N_CORES)), **kw)

    out = np.concatenate([res.results[k]["out"] for k in range(N_CORES)], axis=0)
    if _profile is not None:
        return out.astype(np.float32), res
    return out.astype(np.float32)



# revision 2
# speedup vs baseline: 2.3754x; 2.3754x over previous
"""Trainium2 Bass kernel for nn_ClockAwareGNN (segment_reduce).

Model (reference, fp32):
    gp   = segment_mean(x, batch) @ W_base + b_base            # [B, 1]
    h    = relu(clock @ W1 + b1) @ W2 + b2                     # [N, 16]
    cp   = segment_mean(h, batch)                              # [B, 16]
    out  = relu([gp | cp] @ W3 + b3) @ W4 + b4                 # [B, 1]

Everything after the segment reductions is affine in per-graph quantities, so
the heavy per-node work collapses to fused segment sums:
    Sx[g] = sum of x rows in graph g           (128 cols, fp8 payload)
    Sr[g] = sum of r rows in graph g           (R cols, fp8 hi + fp8 lo*512)
where r is either the raw clock (R=1; exact when b1 == 0 and clock >= 0 since
relu(c*W1) == c*relu(W1) elementwise for c >= 0) or the host-computed
relu(clock @ W1 + b1) (R=16 fallback). Graph node counts come from `batch` on
the host (they are index metadata), shipped as a per-graph 1/cnt constant.

Device strategy (per core, 8-way data-parallel by graph):
  - nodes arrive as 128-row tiles; batch ids are sorted so each 32-graph
    "window" owns a contiguous, per-window padded run of tiles.
  - the whole payload is fp8e4m3: x to ~2^-4 relative (segment-mean averages
    the quantization noise down by ~sqrt(n), n~2000) and clock as hi + lo*512
    pair; measured end-to-end rel err ~2.4e-3 vs the 2e-2 gate.
  - DVE builds one-hot assign tiles [128 nodes, 32 graphs] for a whole
    super-tile in one is_equal op (broadcast AP vs an iota pattern).
  - PE accumulates assign.T @ payload into PSUM [128 graphs, C] fp32 with ONE
    matmul per node-tile. Tiles are interleaved across the 4 windows so
    consecutive matmuls land in different PE column groups (tile_position)
    and overlap in the array.
  - tiny vector-engine epilogue computes the folded per-graph MLP.
"""

import math
import sys
import types

import numpy as np
import ml_dtypes

import concourse.bass as bass
import concourse.bacc as bacc
import concourse.tile as tile
from concourse import mybir
from concourse.bass_utils import run_bass_kernel_spmd


def _ensure_axon_hooks():
    """bass_utils' trace path does `from antenv.axon_hooks import ...`;
    some agent images lack that submodule. Install it (with the real NTFF
    hook when available) so trace=True degrades gracefully instead of
    raising ModuleNotFoundError."""
    try:
        import antenv  # noqa: F401
        import antenv.axon_hooks  # noqa: F401
        return
    except ImportError:
        pass
    try:
        import antenv
    except ImportError:
        return
    mod = types.ModuleType("antenv.axon_hooks")
    state = {"hook": None}
    mod.set_axon_ntff_profile_hook = lambda h: state.__setitem__("hook", h)
    mod.get_axon_ntff_profile_hook = lambda: state["hook"]
    sys.modules["antenv.axon_hooks"] = mod
    antenv.axon_hooks = mod
    try:
        from trn_agent_boot.trn_boot import _ntff_profile_via_ctypes
        mod.set_axon_ntff_profile_hook(
            _ntff_profile_via_ctypes("/opt/axon/libaxon_pjrt.so"))
    except Exception:
        pass
    # the trace path also uploads the NEFF dir to a bucket; in zero-egress
    # containers that raises — fall back to the local path.
    try:
        import concourse.bass_utils as _bu
        _orig_upload = _bu.upload_artifacts

        def _safe_upload(tmpdir):
            try:
                return _orig_upload(tmpdir)
            except Exception:
                return str(tmpdir)

        _bu.upload_artifacts = _safe_upload
    except Exception:
        pass


_ensure_axon_hooks()

BF16 = ml_dtypes.bfloat16
F8 = ml_dtypes.float8_e4m3

N_CORES = 8
N_GRAPHS = 1024
D = 128                 # feature dim of x
GPC = N_GRAPHS // N_CORES   # graphs per core = 128
W = 32                  # one-hot window width (PSUM partition alignment unit)
WPC = GPC // W          # windows per core = 4
ST = 64                 # node-tiles per DMA super-tile (ST % WPC == 0)
LO_SCALE = 512.0        # fp8 lo-correction pre-scale (2^9)


def _build_program(S, C, R):
    """Build the SPMD Bass/Tile program. Shapes are static; per-core data
    differences live entirely in the input tensors.

    S: number of super-tiles (each ST node-tiles of 128 nodes)
    C: fp8 payload column count = 128 + 2*R
    """
    fp32 = mybir.dt.float32
    bf16 = mybir.dt.bfloat16
    f8 = mybir.dt.float8e4
    n_tiles = S * ST

    nc = bacc.Bacc("TRN2", target_bir_lowering=False, debug=False,
                   num_devices=N_CORES)

    xcc = nc.dram_tensor("xcc", [S, 128, ST * C], f8, kind="ExternalInput").ap()
    brs = nc.dram_tensor("brs", [128, S * ST], bf16, kind="ExternalInput").ap()
    iota_c = nc.dram_tensor("iota_c", [128, ST * W], bf16, kind="ExternalInput").ap()
    wbase_b = nc.dram_tensor("wbase_b", [128, D], fp32, kind="ExternalInput").ap()
    v1_b = nc.dram_tensor("v1_b", [128, 32], fp32, kind="ExternalInput").ap()
    m2_b = nc.dram_tensor("m2_b", [128, R * 32], fp32, kind="ExternalInput").ap()
    v0_b = nc.dram_tensor("v0_b", [128, 32], fp32, kind="ExternalInput").ap()
    w4_b = nc.dram_tensor("w4_b", [128, 32], fp32, kind="ExternalInput").ap()
    bb_t = nc.dram_tensor("bb_t", [128, 1], fp32, kind="ExternalInput").ap()
    b4_t = nc.dram_tensor("b4_t", [128, 1], fp32, kind="ExternalInput").ap()
    rec_b = nc.dram_tensor("rec_b", [128, 1], fp32, kind="ExternalInput").ap()
    out_d = nc.dram_tensor("out", [128, 1], fp32, kind="ExternalOutput").ap()

    with tile.TileContext(nc) as tc:
        with (
            tc.tile_pool(name="consts", bufs=1) as cpool,
            tc.tile_pool(name="xin", bufs=8) as xpool,
            tc.tile_pool(name="assign", bufs=4) as apool,
            tc.tile_pool(name="epi", bufs=1) as epool,
            tc.tile_pool(name="ps", bufs=1, space="PSUM") as ppool,
        ):
            # ---- constants ----
            iota_t = cpool.tile([128, ST * W], bf16, tag="iota")
            nc.sync.dma_start(iota_t[:], iota_c)
            # whole-run batch-rel ids: one small DMA instead of one per super
            brall = cpool.tile([128, S * ST], bf16, tag="brall")
            nc.sync.dma_start(brall[:], brs)
            wb_t = cpool.tile([128, D], fp32, tag="wb")
            nc.sync.dma_start(wb_t[:], wbase_b)
            v1_t = cpool.tile([128, 32], fp32, tag="v1")
            nc.sync.dma_start(v1_t[:], v1_b)
            m2_t = cpool.tile([128, R * 32], fp32, tag="m2")
            nc.sync.dma_start(m2_t[:], m2_b)
            v0_t = cpool.tile([128, 32], fp32, tag="v0")
            nc.sync.dma_start(v0_t[:], v0_b)
            w4_t = cpool.tile([128, 32], fp32, tag="w4")
            nc.sync.dma_start(w4_t[:], w4_b)
            bbt = cpool.tile([128, 1], fp32, tag="bb")
            nc.sync.dma_start(bbt[:], bb_t)
            b4t = cpool.tile([128, 1], fp32, tag="b4")
            nc.sync.dma_start(b4t[:], b4_t)
            rect = cpool.tile([128, 1], fp32, tag="rec")
            nc.sync.dma_start(rect[:], rec_b)

            psum = ppool.tile([128, C], fp32, tag="acc")

            # init matmul: zero weights x zero rhs, start=True claims the
            # whole bank's has_written bits so all later matmuls (start=False)
            # overwrite-on-first-touch / accumulate-after, independent of
            # window interleaving.
            zw = cpool.tile([128, 128], bf16, tag="zw")
            nc.vector.memset(zw[:], 0.0)
            zr = cpool.tile([128, C], bf16, tag="zr")
            nc.vector.memset(zr[:], 0.0)
            nc.tensor.matmul(psum[:, :], zw[:], zr[:], start=True, stop=False)

            # ---- main loop ----
            for s in range(S):
                xt = xpool.tile([128, ST * C], f8, tag="xt")
                # alternate the two HWDGE rings (SP / ACT) between supers
                eng = nc.sync if (s % 2 == 0) else nc.scalar
                eng.dma_start(xt[:], xcc[s])
                # one-hot assign for all ST tiles in one DVE op:
                # asg[p, t, j] = (iota[j] == br[p, s*ST + t])
                asg = apool.tile([128, ST * W], bf16, tag="asg")
                nc.vector.tensor_tensor(
                    asg[:].rearrange("p (t j) -> p t j", j=W),
                    iota_t[:].rearrange("p (t j) -> p t j", j=W),
                    brall[:, s * ST : (s + 1) * ST]
                        .rearrange("p (t o) -> p t o", o=1)
                        .to_broadcast((128, ST, W)),
                    op=mybir.AluOpType.is_equal,
                )
                for t in range(ST):
                    i = s * ST + t
                    w = i % WPC         # column-group interleave across windows
                    last = i == n_tiles - 1
                    nc.tensor.matmul(
                        psum[w * W : (w + 1) * W, 0:C],
                        asg[:, t * W : (t + 1) * W],
                        xt[:, t * C : (t + 1) * C],
                        start=False,
                        stop=last,
                        tile_position=(0, w * W),
                    )

            # ---- epilogue (per-graph folded MLP) ----
            sb = epool.tile([128, C], fp32, tag="sb")
            nc.vector.tensor_copy(sb[:], psum[:])

            # Sr = hi_sums + lo_sums / LO_SCALE
            slo = epool.tile([128, R], fp32, tag="slo")
            nc.vector.tensor_scalar_mul(slo[:], sb[:, D + R : D + 2 * R], 1.0 / LO_SCALE)
            sr = epool.tile([128, R], fp32, tag="sr")
            nc.vector.tensor_add(sr[:], sb[:, D : D + R], slo[:])

            mx = epool.tile([128, D], fp32, tag="mx")
            nc.vector.tensor_scalar_mul(mx[:], sb[:, 0:D], rect[:])
            mr = epool.tile([128, R], fp32, tag="mr")
            nc.vector.tensor_scalar_mul(mr[:], sr[:], rect[:])

            # gp = rowsum(mean_x * W_base) + b_base
            t1 = epool.tile([128, D], fp32, tag="t1")
            nc.vector.tensor_mul(t1[:], mx[:], wb_t[:])
            gp = epool.tile([128, 1], fp32, tag="gp")
            nc.vector.tensor_reduce(gp[:], t1[:], axis=mybir.AxisListType.X,
                                    op=mybir.AluOpType.add)
            nc.vector.tensor_add(gp[:], gp[:], bbt[:])

            # pre = gp*v1 + sum_j mr[:,j]*M2[j] + v0
            pre = epool.tile([128, 32], fp32, tag="pre")
            nc.vector.tensor_scalar_mul(pre[:], v1_t[:], gp[:])
            tmp = epool.tile([128, 32], fp32, tag="tmp")
            for j in range(R):
                nc.vector.tensor_scalar(
                    tmp[:], m2_t[:, j * 32 : (j + 1) * 32], mr[:, j : j + 1], None,
                    op0=mybir.AluOpType.mult,
                )
                nc.vector.tensor_add(pre[:], pre[:], tmp[:])
            nc.vector.tensor_add(pre[:], pre[:], v0_t[:])

            act = epool.tile([128, 32], fp32, tag="act")
            nc.scalar.activation(act[:], pre[:], mybir.ActivationFunctionType.Relu)

            # out = rowsum(act * W4) + b4
            nc.vector.tensor_mul(act[:], act[:], w4_t[:])
            oo = epool.tile([128, 1], fp32, tag="oo")
            nc.vector.tensor_reduce(oo[:], act[:], axis=mybir.AxisListType.X,
                                    op=mybir.AluOpType.add)
            nc.vector.tensor_add(oo[:], oo[:], b4t[:])

            nc.sync.dma_start(out_d, oo[:])

    nc.compile()
    return nc


def kernel(x, clock_period, batch, W_base, b_base, W1, b1, W2, b2, W3, b3, W4, b4,
           _profile=None):
    x = np.asarray(x, np.float32)
    clock = np.asarray(clock_period, np.float32).reshape(-1)
    batch = np.asarray(batch, np.int32)
    W_base = np.asarray(W_base, np.float32)
    W1 = np.asarray(W1, np.float32); b1 = np.asarray(b1, np.float32)
    W2 = np.asarray(W2, np.float32); b2 = np.asarray(b2, np.float32)
    W3 = np.asarray(W3, np.float32); b3 = np.asarray(b3, np.float32)
    W4 = np.asarray(W4, np.float32); b4 = np.asarray(b4, np.float32)
    hid = W1.shape[1]

    # r-path: exact algebraic fold when relu(c*W1 + b1) == c * relu(W1)
    fold = bool(np.all(b1 == 0.0)) and bool(clock.min() >= 0.0)
    if fold:
        R = 1
        r32 = clock[:, None]                                   # [N, 1]
        q = np.maximum(W1, 0.0) @ W2                           # [1, hid]
        M2 = q @ W3[1:, :]                                     # [1, 32]
        v0 = b2 @ W3[1:, :] + b3                               # [32]
    else:
        R = hid
        r32 = np.maximum(clock[:, None] @ W1 + b1, 0.0)        # [N, hid]
        M2 = W2 @ W3[1:, :]                                    # [hid, 32]
        v0 = b2 @ W3[1:, :] + b3

    C = D + 2 * R           # [x | r_hi | r_lo], all fp8e4m3

    # ---- shard by graph; window padding so tile->window map is static ----
    cut = np.searchsorted(batch, np.arange(0, N_GRAPHS + 1, W))
    T_w = int(math.ceil(np.diff(cut).max() / 128.0))
    tpw = ST // WPC         # tiles of one window inside one super-tile
    while T_w % tpw:
        T_w += 1
    n_tiles = WPC * T_w
    S = n_tiles // ST

    gcut = np.searchsorted(batch, np.arange(0, N_GRAPHS + 1))
    cnt = np.diff(gcut).astype(np.float32)
    rec_all = (1.0 / np.maximum(cnt, 1.0)).astype(np.float32)

    x8 = x.astype(F8)
    rhi = r32.astype(F8)
    rlo = ((r32 - rhi.astype(np.float32)) * LO_SCALE).astype(F8)

    in_maps = []
    # shared constant tiles
    iota_c = np.broadcast_to(
        np.tile(np.arange(W, dtype=BF16), ST)[None, :], (128, ST * W)
    ).copy()
    wbase_b = np.broadcast_to(W_base[:, 0][None, :], (128, D)).astype(np.float32).copy()
    v1_b = np.broadcast_to(W3[0, :][None, :], (128, 32)).astype(np.float32).copy()
    m2_b = np.broadcast_to(M2.reshape(-1)[None, :], (128, R * 32)).astype(np.float32).copy()
    v0_b = np.broadcast_to(v0[None, :], (128, 32)).astype(np.float32).copy()
    w4_b = np.broadcast_to(W4[:, 0][None, :], (128, 32)).astype(np.float32).copy()
    bb_t = np.full((128, 1), float(b_base.reshape(-1)[0]), np.float32)
    b4_t = np.full((128, 1), float(b4.reshape(-1)[0]), np.float32)

    for k in range(N_CORES):
        wx = np.zeros((WPC, T_w * 128, C), F8)
        wbr = np.full((WPC, T_w * 128), -1.0, BF16)
        for wi in range(WPC):
            gw = k * WPC + wi          # global window index
            s0, e0 = int(cut[gw]), int(cut[gw + 1])
            n = e0 - s0
            wx[wi, :n, 0:D] = x8[s0:e0]
            wx[wi, :n, D : D + R] = rhi[s0:e0]
            wx[wi, :n, D + R : D + 2 * R] = rlo[s0:e0]
            wbr[wi, :n] = (batch[s0:e0] - gw * W).astype(BF16)
        # window-interleaved tile order: tile i = s*ST + t belongs to window
        # i % WPC at within-window slot i // WPC; each SBUF partition line is
        # contiguous in DRAM.
        xcc_p = np.ascontiguousarray(
            wx.reshape(WPC, S, tpw, 128, C).transpose(1, 3, 2, 0, 4)
        ).reshape(S, 128, ST * C)
        brs_p = np.ascontiguousarray(
            wbr.reshape(WPC, S, tpw, 128).transpose(3, 1, 2, 0)
        ).reshape(128, S * ST)
        rec_b = rec_all[k * GPC : (k + 1) * GPC].reshape(128, 1).copy()
        in_maps.append(dict(
            xcc=xcc_p, brs=brs_p, iota_c=iota_c,
            wbase_b=wbase_b, v1_b=v1_b, m2_b=m2_b, v0_b=v0_b, w4_b=w4_b,
            bb_t=bb_t, b4_t=b4_t, rec_b=rec_b,
        ))

    nc = _build_program(S, C, R)

    kw = {}
    if _profile is not None:
        kw = dict(trace=True, **_profile)
    res = run_bass_kernel_spmd(nc, in_maps, list(range(N_CORES)), **kw)

    out = np.concatenate([res.results[k]["out"] for k in range(N_CORES)], axis=0)
    if _profile is not None:
        return out.astype(np.float32), res
    return out.astype(np.float32)


# revision 5
# speedup vs baseline: 2.3776x; 1.0009x over previous
"""Trainium2 Bass kernel for nn_ClockAwareGNN (segment_reduce).

Model (reference, fp32):
    gp   = segment_mean(x, batch) @ W_base + b_base            # [B, 1]
    h    = relu(clock @ W1 + b1) @ W2 + b2                     # [N, 16]
    cp   = segment_mean(h, batch)                              # [B, 16]
    out  = relu([gp | cp] @ W3 + b3) @ W4 + b4                 # [B, 1]

Everything after the segment reductions is affine in per-graph quantities, so
the heavy per-node work collapses to fused segment sums:
    Sx[g] = sum of x rows in graph g           (128 cols, fp8 payload)
    Sr[g] = sum of r rows in graph g           (R cols, fp8 hi + fp8 lo*512)
where r is either the raw clock (R=1; exact when b1 == 0 and clock >= 0 since
relu(c*W1) == c*relu(W1) elementwise for c >= 0) or the host-computed
relu(clock @ W1 + b1) (R=16 fallback). Graph node counts come from `batch` on
the host (they are index metadata), shipped as a per-graph 1/cnt constant.

Device strategy (per core, 8-way data-parallel by graph):
  - nodes arrive as 128-row tiles; batch ids are sorted so each 32-graph
    "window" owns a contiguous, per-window padded run of tiles.
  - the whole payload is fp8e4m3: x to ~2^-4 relative (segment-mean averages
    the quantization noise down by ~sqrt(n), n~2000) and clock as hi + lo*512
    pair; measured end-to-end rel err ~2.4e-3 vs the 2e-2 gate.
  - DVE builds one-hot assign tiles [128 nodes, 32 graphs] for a whole
    super-tile in one is_equal op (broadcast AP vs an iota pattern).
  - PE accumulates assign.T @ payload into PSUM [128 graphs, C] fp32 with ONE
    matmul per node-tile. Tiles are interleaved across the 4 windows so
    consecutive matmuls land in different PE column groups (tile_position)
    and overlap in the array.
  - tiny vector-engine epilogue computes the folded per-graph MLP.
"""

import math
import sys
import types

import numpy as np
import ml_dtypes

import concourse.bass as bass
import concourse.bacc as bacc
import concourse.tile as tile
from concourse import mybir
from concourse.bass_utils import run_bass_kernel_spmd


def _ensure_axon_hooks():
    """bass_utils' trace path does `from antenv.axon_hooks import ...`;
    some agent images lack that submodule. Install it (with the real NTFF
    hook when available) so trace=True degrades gracefully instead of
    raising ModuleNotFoundError."""
    try:
        import antenv  # noqa: F401
        import antenv.axon_hooks  # noqa: F401
        return
    except ImportError:
        pass
    try:
        import antenv
    except ImportError:
        return
    mod = types.ModuleType("antenv.axon_hooks")
    state = {"hook": None}
    mod.set_axon_ntff_profile_hook = lambda h: state.__setitem__("hook", h)
    mod.get_axon_ntff_profile_hook = lambda: state["hook"]
    sys.modules["antenv.axon_hooks"] = mod
    antenv.axon_hooks = mod
    try:
        from trn_agent_boot.trn_boot import _ntff_profile_via_ctypes
        mod.set_axon_ntff_profile_hook(
            _ntff_profile_via_ctypes("/opt/axon/libaxon_pjrt.so"))
    except Exception:
        pass
    # the trace path also uploads the NEFF dir to a bucket; in zero-egress
    # containers that raises — fall back to the local path.
    try:
        import concourse.bass_utils as _bu
        _orig_upload = _bu.upload_artifacts

        def _safe_upload(tmpdir):
            try:
                return _orig_upload(tmpdir)
            except Exception:
                return str(tmpdir)

        _bu.upload_artifacts = _safe_upload
    except Exception:
        pass


_ensure_axon_hooks()

BF16 = ml_dtypes.bfloat16
F8 = ml_dtypes.float8_e4m3

N_CORES = 8
N_GRAPHS = 1024
D = 128                 # feature dim of x
GPC = N_GRAPHS // N_CORES   # graphs per core = 128
W = 32                  # one-hot window width (PSUM partition alignment unit)
WPC = GPC // W          # windows per core = 4
ST = 64                 # node-tiles per DMA super-tile (ST % WPC == 0)
LO_SCALE = 512.0        # fp8 lo-correction pre-scale (2^9)


def _build_program(S, C, R):
    """Build the SPMD Bass/Tile program. Shapes are static; per-core data
    differences live entirely in the input tensors.

    S: number of super-tiles (each ST node-tiles of 128 nodes)
    C: fp8 payload column count = 128 + 2*R
    """
    fp32 = mybir.dt.float32
    bf16 = mybir.dt.bfloat16
    f8 = mybir.dt.float8e4
    n_tiles = S * ST

    nc = bacc.Bacc("TRN2", target_bir_lowering=False, debug=False,
                   num_devices=N_CORES)

    xcc = nc.dram_tensor("xcc", [S, 128, ST * C], f8, kind="ExternalInput").ap()
    brs = nc.dram_tensor("brs", [128, S * ST], bf16, kind="ExternalInput").ap()
    iota_c = nc.dram_tensor("iota_c", [128, ST * W], bf16, kind="ExternalInput").ap()
    wbase_b = nc.dram_tensor("wbase_b", [128, D], fp32, kind="ExternalInput").ap()
    v1_b = nc.dram_tensor("v1_b", [128, 32], fp32, kind="ExternalInput").ap()
    m2_b = nc.dram_tensor("m2_b", [128, R * 32], fp32, kind="ExternalInput").ap()
    v0_b = nc.dram_tensor("v0_b", [128, 32], fp32, kind="ExternalInput").ap()
    w4_b = nc.dram_tensor("w4_b", [128, 32], fp32, kind="ExternalInput").ap()
    bb_t = nc.dram_tensor("bb_t", [128, 1], fp32, kind="ExternalInput").ap()
    b4_t = nc.dram_tensor("b4_t", [128, 1], fp32, kind="ExternalInput").ap()
    rec_b = nc.dram_tensor("rec_b", [128, 1], fp32, kind="ExternalInput").ap()
    out_d = nc.dram_tensor("out", [128, 1], fp32, kind="ExternalOutput").ap()

    with tile.TileContext(nc) as tc:
        with (
            tc.tile_pool(name="consts", bufs=1) as cpool,
            tc.tile_pool(name="xin", bufs=5) as xpool,
            tc.tile_pool(name="assign", bufs=1) as apool,
            tc.tile_pool(name="epi", bufs=1) as epool,
            tc.tile_pool(name="ps", bufs=1, space="PSUM") as ppool,
        ):
            # ---- constants ----
            iota_t = cpool.tile([128, ST * W], bf16, tag="iota")
            nc.sync.dma_start(iota_t[:], iota_c)
            # whole-run batch-rel ids: one small DMA instead of one per super
            brall = cpool.tile([128, S * ST], bf16, tag="brall")
            nc.sync.dma_start(brall[:], brs)
            wb_t = cpool.tile([128, D], fp32, tag="wb")
            nc.sync.dma_start(wb_t[:], wbase_b)
            v1_t = cpool.tile([128, 32], fp32, tag="v1")
            nc.sync.dma_start(v1_t[:], v1_b)
            m2_t = cpool.tile([128, R * 32], fp32, tag="m2")
            nc.sync.dma_start(m2_t[:], m2_b)
            v0_t = cpool.tile([128, 32], fp32, tag="v0")
            nc.sync.dma_start(v0_t[:], v0_b)
            w4_t = cpool.tile([128, 32], fp32, tag="w4")
            nc.sync.dma_start(w4_t[:], w4_b)
            bbt = cpool.tile([128, 1], fp32, tag="bb")
            nc.sync.dma_start(bbt[:], bb_t)
            b4t = cpool.tile([128, 1], fp32, tag="b4")
            nc.sync.dma_start(b4t[:], b4_t)
            rect = cpool.tile([128, 1], fp32, tag="rec")
            nc.sync.dma_start(rect[:], rec_b)

            psum = ppool.tile([128, C], fp32, tag="acc")

            # init matmul: zero weights x zero rhs, start=True claims the
            # whole bank's has_written bits so all later matmuls (start=False)
            # overwrite-on-first-touch / accumulate-after, independent of
            # window interleaving.
            zw = cpool.tile([128, 128], bf16, tag="zw")
            nc.vector.memset(zw[:], 0.0)
            zr = cpool.tile([128, C], bf16, tag="zr")
            nc.vector.memset(zr[:], 0.0)
            nc.tensor.matmul(psum[:, :], zw[:], zr[:], start=True, stop=False)

            # ---- one-hot assign tiles, built up-front off the critical path:
            # they depend only on constants, so the DVE runs ahead of the
            # DMA/PE pipeline instead of pacing it per super-tile.
            # asg[p, t, j] = (iota[j] == br[p, s*ST + t])
            asgs = []
            for s in range(S):
                asg = apool.tile([128, ST * W], bf16, tag=f"asg{s}")
                nc.vector.tensor_tensor(
                    asg[:].rearrange("p (t j) -> p t j", j=W),
                    iota_t[:].rearrange("p (t j) -> p t j", j=W),
                    brall[:, s * ST : (s + 1) * ST]
                        .rearrange("p (t o) -> p t o", o=1)
                        .to_broadcast((128, ST, W)),
                    op=mybir.AluOpType.is_equal,
                )
                asgs.append(asg)

            # ---- main loop ----
            for s in range(S):
                xt = xpool.tile([128, ST * C], f8, tag="xt")
                # alternate the two HWDGE rings (SP / ACT) between supers
                eng = nc.sync if (s % 2 == 0) else nc.scalar
                eng.dma_start(xt[:], xcc[s])
                asg = asgs[s]
                for t in range(ST):
                    i = s * ST + t
                    w = i % WPC         # column-group interleave across windows
                    last = i == n_tiles - 1
                    nc.tensor.matmul(
                        psum[w * W : (w + 1) * W, 0:C],
                        asg[:, t * W : (t + 1) * W],
                        xt[:, t * C : (t + 1) * C],
                        start=False,
                        stop=last,
                        tile_position=(0, w * W),
                    )

            # ---- epilogue (per-graph folded MLP) ----
            sb = epool.tile([128, C], fp32, tag="sb")
            nc.vector.tensor_copy(sb[:], psum[:])

            # Sr = hi_sums + lo_sums / LO_SCALE
            slo = epool.tile([128, R], fp32, tag="slo")
            nc.vector.tensor_scalar_mul(slo[:], sb[:, D + R : D + 2 * R], 1.0 / LO_SCALE)
            sr = epool.tile([128, R], fp32, tag="sr")
            nc.vector.tensor_add(sr[:], sb[:, D : D + R], slo[:])

            mx = epool.tile([128, D], fp32, tag="mx")
            nc.vector.tensor_scalar_mul(mx[:], sb[:, 0:D], rect[:])
            mr = epool.tile([128, R], fp32, tag="mr")
            nc.vector.tensor_scalar_mul(mr[:], sr[:], rect[:])

            # gp = rowsum(mean_x * W_base) + b_base
            t1 = epool.tile([128, D], fp32, tag="t1")
            nc.vector.tensor_mul(t1[:], mx[:], wb_t[:])
            gp = epool.tile([128, 1], fp32, tag="gp")
            nc.vector.tensor_reduce(gp[:], t1[:], axis=mybir.AxisListType.X,
                                    op=mybir.AluOpType.add)
            nc.vector.tensor_add(gp[:], gp[:], bbt[:])

            # pre = gp*v1 + sum_j mr[:,j]*M2[j] + v0
            pre = epool.tile([128, 32], fp32, tag="pre")
            nc.vector.tensor_scalar_mul(pre[:], v1_t[:], gp[:])
            tmp = epool.tile([128, 32], fp32, tag="tmp")
            for j in range(R):
                nc.vector.tensor_scalar(
                    tmp[:], m2_t[:, j * 32 : (j + 1) * 32], mr[:, j : j + 1], None,
                    op0=mybir.AluOpType.mult,
                )
                nc.vector.tensor_add(pre[:], pre[:], tmp[:])
            nc.vector.tensor_add(pre[:], pre[:], v0_t[:])

            act = epool.tile([128, 32], fp32, tag="act")
            nc.scalar.activation(act[:], pre[:], mybir.ActivationFunctionType.Relu)

            # out = rowsum(act * W4) + b4
            nc.vector.tensor_mul(act[:], act[:], w4_t[:])
            oo = epool.tile([128, 1], fp32, tag="oo")
            nc.vector.tensor_reduce(oo[:], act[:], axis=mybir.AxisListType.X,
                                    op=mybir.AluOpType.add)
            nc.vector.tensor_add(oo[:], oo[:], b4t[:])

            nc.sync.dma_start(out_d, oo[:])

    nc.compile()
    return nc


def kernel(x, clock_period, batch, W_base, b_base, W1, b1, W2, b2, W3, b3, W4, b4,
           _profile=None):
    x = np.asarray(x, np.float32)
    clock = np.asarray(clock_period, np.float32).reshape(-1)
    batch = np.asarray(batch, np.int32)
    W_base = np.asarray(W_base, np.float32)
    W1 = np.asarray(W1, np.float32); b1 = np.asarray(b1, np.float32)
    W2 = np.asarray(W2, np.float32); b2 = np.asarray(b2, np.float32)
    W3 = np.asarray(W3, np.float32); b3 = np.asarray(b3, np.float32)
    W4 = np.asarray(W4, np.float32); b4 = np.asarray(b4, np.float32)
    hid = W1.shape[1]

    # r-path: exact algebraic fold when relu(c*W1 + b1) == c * relu(W1)
    fold = bool(np.all(b1 == 0.0)) and bool(clock.min() >= 0.0)
    if fold:
        R = 1
        r32 = clock[:, None]                                   # [N, 1]
        q = np.maximum(W1, 0.0) @ W2                           # [1, hid]
        M2 = q @ W3[1:, :]                                     # [1, 32]
        v0 = b2 @ W3[1:, :] + b3                               # [32]
    else:
        R = hid
        r32 = np.maximum(clock[:, None] @ W1 + b1, 0.0)        # [N, hid]
        M2 = W2 @ W3[1:, :]                                    # [hid, 32]
        v0 = b2 @ W3[1:, :] + b3

    C = D + 2 * R           # [x | r_hi | r_lo], all fp8e4m3

    # ---- shard by graph; window padding so tile->window map is static ----
    cut = np.searchsorted(batch, np.arange(0, N_GRAPHS + 1, W))
    T_w = int(math.ceil(np.diff(cut).max() / 128.0))
    tpw = ST // WPC         # tiles of one window inside one super-tile
    while T_w % tpw:
        T_w += 1
    n_tiles = WPC * T_w
    S = n_tiles // ST

    gcut = np.searchsorted(batch, np.arange(0, N_GRAPHS + 1))
    cnt = np.diff(gcut).astype(np.float32)
    rec_all = (1.0 / np.maximum(cnt, 1.0)).astype(np.float32)

    x8 = x.astype(F8)
    rhi = r32.astype(F8)
    rlo = ((r32 - rhi.astype(np.float32)) * LO_SCALE).astype(F8)

    in_maps = []
    # shared constant tiles
    iota_c = np.broadcast_to(
        np.tile(np.arange(W, dtype=BF16), ST)[None, :], (128, ST * W)
    ).copy()
    wbase_b = np.broadcast_to(W_base[:, 0][None, :], (128, D)).astype(np.float32).copy()
    v1_b = np.broadcast_to(W3[0, :][None, :], (128, 32)).astype(np.float32).copy()
    m2_b = np.broadcast_to(M2.reshape(-1)[None, :], (128, R * 32)).astype(np.float32).copy()
    v0_b = np.broadcast_to(v0[None, :], (128, 32)).astype(np.float32).copy()
    w4_b = np.broadcast_to(W4[:, 0][None, :], (128, 32)).astype(np.float32).copy()
    bb_t = np.full((128, 1), float(b_base.reshape(-1)[0]), np.float32)
    b4_t = np.full((128, 1), float(b4.reshape(-1)[0]), np.float32)

    for k in range(N_CORES):
        wx = np.zeros((WPC, T_w * 128, C), F8)
        wbr = np.full((WPC, T_w * 128), -1.0, BF16)
        for wi in range(WPC):
            gw = k * WPC + wi          # global window index
            s0, e0 = int(cut[gw]), int(cut[gw + 1])
            n = e0 - s0
            wx[wi, :n, 0:D] = x8[s0:e0]
            wx[wi, :n, D : D + R] = rhi[s0:e0]
            wx[wi, :n, D + R : D + 2 * R] = rlo[s0:e0]
            wbr[wi, :n] = (batch[s0:e0] - gw * W).astype(BF16)
        # window-interleaved tile order: tile i = s*ST + t belongs to window
        # i % WPC at within-window slot i // WPC; each SBUF partition line is
        # contiguous in DRAM.
        xcc_p = np.ascontiguousarray(
            wx.reshape(WPC, S, tpw, 128, C).transpose(1, 3, 2, 0, 4)
        ).reshape(S, 128, ST * C)
        brs_p = np.ascontiguousarray(
            wbr.reshape(WPC, S, tpw, 128).transpose(3, 1, 2, 0)
        ).reshape(128, S * ST)
        rec_b = rec_all[k * GPC : (k + 1) * GPC].reshape(128, 1).copy()
        in_maps.append(dict(
            xcc=xcc_p, brs=brs_p, iota_c=iota_c,
            wbase_b=wbase_b, v1_b=v1_b, m2_b=m2_b, v0_b=v0_b, w4_b=w4_b,
            bb_t=bb_t, b4_t=b4_t, rec_b=rec_b,
        ))

    nc = _build_program(S, C, R)

    kw = {}
    if _profile is not None:
        kw = dict(trace=True, **_profile)
    res = run_bass_kernel_spmd(nc, in_maps, list(range(N_CORES)), **kw)

    out = np.concatenate([res.results[k]["out"] for k in range(N_CORES)], axis=0)
    if _profile is not None:
        return out.astype(np.float32), res
    return out.astype(np.float32)


# revision 10
# speedup vs baseline: 2.4278x; 1.0211x over previous
"""Trainium2 Bass kernel for nn_ClockAwareGNN (segment_reduce).

Model (reference, fp32):
    gp   = segment_mean(x, batch) @ W_base + b_base            # [B, 1]
    h    = relu(clock @ W1 + b1) @ W2 + b2                     # [N, 16]
    cp   = segment_mean(h, batch)                              # [B, 16]
    out  = relu([gp | cp] @ W3 + b3) @ W4 + b4                 # [B, 1]

Everything after the segment reductions is affine in per-graph quantities, so
the heavy per-node work collapses to fused segment sums:
    Sx[g] = sum of x rows in graph g           (128 cols, fp8 payload)
    Sr[g] = sum of r rows in graph g           (R cols, fp8 hi + fp8 lo*512)
where r is either the raw clock (R=1; exact when b1 == 0 and clock >= 0 since
relu(c*W1) == c*relu(W1) elementwise for c >= 0) or the host-computed
relu(clock @ W1 + b1) (R=16 fallback). Graph node counts come from `batch` on
the host (they are index metadata), shipped as a per-graph 1/cnt constant.

Device strategy (per core, 8-way data-parallel by graph):
  - nodes arrive as 128-row tiles; batch ids are sorted so each 32-graph
    "window" owns a contiguous, per-window padded run of tiles.
  - the whole payload is fp8e4m3: x to ~2^-4 relative (segment-mean averages
    the quantization noise down by ~sqrt(n), n~2000) and clock as hi + lo*512
    pair; measured end-to-end rel err ~2.4e-3 vs the 2e-2 gate.
  - DVE builds one-hot assign tiles [128 nodes, 32 graphs] for a whole
    super-tile in one is_equal op (broadcast AP vs an iota pattern).
  - PE accumulates assign.T @ payload into PSUM [128 graphs, C] fp32 with ONE
    matmul per node-tile. Tiles are interleaved across the 4 windows so
    consecutive matmuls land in different PE column groups (tile_position)
    and overlap in the array.
  - tiny vector-engine epilogue computes the folded per-graph MLP.
"""

import math
import sys
import types

import numpy as np
import ml_dtypes

import concourse.bass as bass
import concourse.bacc as bacc
import concourse.tile as tile
from concourse import mybir
from concourse.bass_utils import run_bass_kernel_spmd


def _ensure_axon_hooks():
    """bass_utils' trace path does `from antenv.axon_hooks import ...`;
    some agent images lack that submodule. Install it (with the real NTFF
    hook when available) so trace=True degrades gracefully instead of
    raising ModuleNotFoundError."""
    try:
        import antenv  # noqa: F401
        import antenv.axon_hooks  # noqa: F401
        return
    except ImportError:
        pass
    try:
        import antenv
    except ImportError:
        return
    mod = types.ModuleType("antenv.axon_hooks")
    state = {"hook": None}
    mod.set_axon_ntff_profile_hook = lambda h: state.__setitem__("hook", h)
    mod.get_axon_ntff_profile_hook = lambda: state["hook"]
    sys.modules["antenv.axon_hooks"] = mod
    antenv.axon_hooks = mod
    try:
        from trn_agent_boot.trn_boot import _ntff_profile_via_ctypes
        mod.set_axon_ntff_profile_hook(
            _ntff_profile_via_ctypes("/opt/axon/libaxon_pjrt.so"))
    except Exception:
        pass
    # the trace path also uploads the NEFF dir to a bucket; in zero-egress
    # containers that raises — fall back to the local path.
    try:
        import concourse.bass_utils as _bu
        _orig_upload = _bu.upload_artifacts

        def _safe_upload(tmpdir):
            try:
                return _orig_upload(tmpdir)
            except Exception:
                return str(tmpdir)

        _bu.upload_artifacts = _safe_upload
    except Exception:
        pass


_ensure_axon_hooks()

BF16 = ml_dtypes.bfloat16
F8 = ml_dtypes.float8_e4m3

N_CORES = 8
N_GRAPHS = 1024
D = 128                 # feature dim of x
GPC = N_GRAPHS // N_CORES   # graphs per core = 128
W = 32                  # one-hot window width (PSUM partition alignment unit)
WPC = GPC // W          # windows per core = 4
ST = 64                 # node-tiles per DMA super-tile (ST % WPC == 0)
LO_SCALE = 512.0        # fp8 lo-correction pre-scale (2^9)


def _build_program(S, C, R):
    """Build the SPMD Bass/Tile program. Shapes are static; per-core data
    differences live entirely in the input tensors.

    S: number of super-tiles (each ST node-tiles of 128 nodes)
    C: fp8 payload column count = 128 + 2*R
    """
    fp32 = mybir.dt.float32
    bf16 = mybir.dt.bfloat16
    f8 = mybir.dt.float8e4
    n_tiles = S * ST

    nc = bacc.Bacc("TRN2", target_bir_lowering=False, debug=False,
                   num_devices=N_CORES)

    # bf16 block: [brall (S*ST) | iota (ST*W)]; fp32 block:
    # [wb (D) | v1 (32) | m2 (R*32) | v0 (32) | w4 (32) | bb | b4 | rec]
    NB = S * ST + ST * W
    NF = D + 32 + R * 32 + 32 + 32 + 3
    xcc = nc.dram_tensor("xcc", [S, 128, ST * C], f8, kind="ExternalInput").ap()
    cb16 = nc.dram_tensor("cb16", [128, NB], bf16, kind="ExternalInput").ap()
    cb32 = nc.dram_tensor("cb32", [128, NF], fp32, kind="ExternalInput").ap()
    out_d = nc.dram_tensor("out", [128, 1], fp32, kind="ExternalOutput").ap()

    with tile.TileContext(nc) as tc:
        with (
            tc.tile_pool(name="consts", bufs=1) as cpool,
            tc.tile_pool(name="xin", bufs=6) as xpool,
            tc.tile_pool(name="assign", bufs=1) as apool,
            tc.tile_pool(name="epi", bufs=1) as epool,
            tc.tile_pool(name="ps", bufs=1, space="PSUM") as ppool,
        ):
            # ---- constants: two batched DMAs instead of eleven small ones ----
            cb16_t = cpool.tile([128, NB], bf16, tag="cb16")
            nc.sync.dma_start(cb16_t[:], cb16)
            cb32_t = cpool.tile([128, NF], fp32, tag="cb32")
            nc.scalar.dma_start(cb32_t[:], cb32)
            brall = cb16_t[:, 0 : S * ST]
            iota_t = cb16_t[:, S * ST : S * ST + ST * W]
            o = 0
            wb_t = cb32_t[:, o : o + D]; o += D
            v1_t = cb32_t[:, o : o + 32]; o += 32
            m2_t = cb32_t[:, o : o + R * 32]; o += R * 32
            v0_t = cb32_t[:, o : o + 32]; o += 32
            w4_t = cb32_t[:, o : o + 32]; o += 32
            bbt = cb32_t[:, o : o + 1]; o += 1
            b4t = cb32_t[:, o : o + 1]; o += 1
            rect = cb32_t[:, o : o + 1]; o += 1

            psum = ppool.tile([128, C], fp32, tag="acc")

            # init matmul: zero weights x zero rhs, start=True claims the
            # whole bank's has_written bits so all later matmuls (start=False)
            # overwrite-on-first-touch / accumulate-after, independent of
            # window interleaving.
            zw = cpool.tile([128, 128], bf16, tag="zw")
            nc.vector.memset(zw[:], 0.0)
            zr = cpool.tile([128, C], bf16, tag="zr")
            nc.vector.memset(zr[:], 0.0)
            nc.tensor.matmul(psum[:, :], zw[:], zr[:], start=True, stop=False)

            # ---- one-hot assign tiles, built up-front off the critical path:
            # they depend only on constants, so the DVE runs ahead of the
            # DMA/PE pipeline instead of pacing it per super-tile.
            # asg[p, t, j] = (iota[j] == br[p, s*ST + t])
            asgs = []
            for s in range(S):
                asg = apool.tile([128, ST * W], bf16, tag=f"asg{s}")
                nc.vector.tensor_tensor(
                    asg[:].rearrange("p (t j) -> p t j", j=W),
                    iota_t.rearrange("p (t j) -> p t j", j=W),
                    brall[:, s * ST : (s + 1) * ST]
                        .rearrange("p (t o) -> p t o", o=1)
                        .to_broadcast((128, ST, W)),
                    op=mybir.AluOpType.is_equal,
                )
                asgs.append(asg)

            # ---- main loop ----
            H = (ST // 2) * C
            for s in range(S):
                xt = xpool.tile([128, ST * C], f8, tag="xt")
                # split each super across BOTH HWDGE rings (SP + ACT) so the
                # two rings stream one super concurrently; subtile deps let
                # the first half's matmuls start before the second half lands
                nc.sync.dma_start(xt[:, 0:H], xcc[s][:, 0:H])
                nc.scalar.dma_start(xt[:, H : 2 * H], xcc[s][:, H : 2 * H])
                asg = asgs[s]
                for t in range(ST):
                    i = s * ST + t
                    w = i % WPC         # column-group interleave across windows
                    last = i == n_tiles - 1
                    nc.tensor.matmul(
                        psum[w * W : (w + 1) * W, 0:C],
                        asg[:, t * W : (t + 1) * W],
                        xt[:, t * C : (t + 1) * C],
                        start=False,
                        stop=last,
                        tile_position=(0, w * W),
                    )

            # ---- epilogue (per-graph folded MLP) ----
            sb = epool.tile([128, C], fp32, tag="sb")
            nc.vector.tensor_copy(sb[:], psum[:])

            # Sr = hi_sums + lo_sums / LO_SCALE
            slo = epool.tile([128, R], fp32, tag="slo")
            nc.vector.tensor_scalar_mul(slo[:], sb[:, D + R : D + 2 * R], 1.0 / LO_SCALE)
            sr = epool.tile([128, R], fp32, tag="sr")
            nc.vector.tensor_add(sr[:], sb[:, D : D + R], slo[:])

            mx = epool.tile([128, D], fp32, tag="mx")
            nc.vector.tensor_scalar_mul(mx[:], sb[:, 0:D], rect)
            mr = epool.tile([128, R], fp32, tag="mr")
            nc.vector.tensor_scalar_mul(mr[:], sr[:], rect)

            # gp = rowsum(mean_x * W_base) + b_base
            t1 = epool.tile([128, D], fp32, tag="t1")
            nc.vector.tensor_mul(t1[:], mx[:], wb_t)
            gp = epool.tile([128, 1], fp32, tag="gp")
            nc.vector.tensor_reduce(gp[:], t1[:], axis=mybir.AxisListType.X,
                                    op=mybir.AluOpType.add)
            nc.vector.tensor_add(gp[:], gp[:], bbt)

            # pre = gp*v1 + sum_j mr[:,j]*M2[j] + v0
            pre = epool.tile([128, 32], fp32, tag="pre")
            nc.vector.tensor_scalar_mul(pre[:], v1_t, gp[:])
            tmp = epool.tile([128, 32], fp32, tag="tmp")
            for j in range(R):
                nc.vector.tensor_scalar(
                    tmp[:], m2_t[:, j * 32 : (j + 1) * 32], mr[:, j : j + 1], None,
                    op0=mybir.AluOpType.mult,
                )
                nc.vector.tensor_add(pre[:], pre[:], tmp[:])
            nc.vector.tensor_add(pre[:], pre[:], v0_t)

            act = epool.tile([128, 32], fp32, tag="act")
            nc.scalar.activation(act[:], pre[:], mybir.ActivationFunctionType.Relu)

            # out = rowsum(act * W4) + b4
            nc.vector.tensor_mul(act[:], act[:], w4_t)
            oo = epool.tile([128, 1], fp32, tag="oo")
            nc.vector.tensor_reduce(oo[:], act[:], axis=mybir.AxisListType.X,
                                    op=mybir.AluOpType.add)
            nc.vector.tensor_add(oo[:], oo[:], b4t)

            nc.sync.dma_start(out_d, oo[:])

    nc.compile()
    return nc


def kernel(x, clock_period, batch, W_base, b_base, W1, b1, W2, b2, W3, b3, W4, b4,
           _profile=None):
    x = np.asarray(x, np.float32)
    clock = np.asarray(clock_period, np.float32).reshape(-1)
    batch = np.asarray(batch, np.int32)
    W_base = np.asarray(W_base, np.float32)
    W1 = np.asarray(W1, np.float32); b1 = np.asarray(b1, np.float32)
    W2 = np.asarray(W2, np.float32); b2 = np.asarray(b2, np.float32)
    W3 = np.asarray(W3, np.float32); b3 = np.asarray(b3, np.float32)
    W4 = np.asarray(W4, np.float32); b4 = np.asarray(b4, np.float32)
    hid = W1.shape[1]

    # r-path: exact algebraic fold when relu(c*W1 + b1) == c * relu(W1)
    fold = bool(np.all(b1 == 0.0)) and bool(clock.min() >= 0.0)
    if fold:
        R = 1
        r32 = clock[:, None]                                   # [N, 1]
        q = np.maximum(W1, 0.0) @ W2                           # [1, hid]
        M2 = q @ W3[1:, :]                                     # [1, 32]
        v0 = b2 @ W3[1:, :] + b3                               # [32]
    else:
        R = hid
        r32 = np.maximum(clock[:, None] @ W1 + b1, 0.0)        # [N, hid]
        M2 = W2 @ W3[1:, :]                                    # [hid, 32]
        v0 = b2 @ W3[1:, :] + b3

    C = D + 2 * R           # [x | r_hi | r_lo], all fp8e4m3

    # ---- shard by graph; window padding so tile->window map is static ----
    cut = np.searchsorted(batch, np.arange(0, N_GRAPHS + 1, W))
    T_w = int(math.ceil(np.diff(cut).max() / 128.0))
    tpw = ST // WPC         # tiles of one window inside one super-tile
    while T_w % tpw:
        T_w += 1
    n_tiles = WPC * T_w
    S = n_tiles // ST

    gcut = np.searchsorted(batch, np.arange(0, N_GRAPHS + 1))
    cnt = np.diff(gcut).astype(np.float32)
    rec_all = (1.0 / np.maximum(cnt, 1.0)).astype(np.float32)

    x8 = x.astype(F8)
    rhi = r32.astype(F8)
    rlo = ((r32 - rhi.astype(np.float32)) * LO_SCALE).astype(F8)

    in_maps = []
    # shared constant blocks
    iota_c = np.tile(np.arange(W, dtype=np.float32), ST)
    cb32_shared = np.concatenate([
        W_base[:, 0], W3[0, :], M2.reshape(-1), v0, W4[:, 0],
        [float(b_base.reshape(-1)[0])], [float(b4.reshape(-1)[0])],
    ]).astype(np.float32)

    for k in range(N_CORES):
        wx = np.zeros((WPC, T_w * 128, C), F8)
        wbr = np.full((WPC, T_w * 128), -1.0, BF16)
        for wi in range(WPC):
            gw = k * WPC + wi          # global window index
            s0, e0 = int(cut[gw]), int(cut[gw + 1])
            n = e0 - s0
            wx[wi, :n, 0:D] = x8[s0:e0]
            wx[wi, :n, D : D + R] = rhi[s0:e0]
            wx[wi, :n, D + R : D + 2 * R] = rlo[s0:e0]
            wbr[wi, :n] = (batch[s0:e0] - gw * W).astype(BF16)
        # window-interleaved tile order: tile i = s*ST + t belongs to window
        # i % WPC at within-window slot i // WPC; each SBUF partition line is
        # contiguous in DRAM.
        xcc_p = np.ascontiguousarray(
            wx.reshape(WPC, S, tpw, 128, C).transpose(1, 3, 2, 0, 4)
        ).reshape(S, 128, ST * C)
        brs_p = np.ascontiguousarray(
            wbr.reshape(WPC, S, tpw, 128).transpose(3, 1, 2, 0)
        ).reshape(128, S * ST)
        cb16_k = np.concatenate(
            [brs_p.astype(BF16),
             np.broadcast_to(iota_c[None, :], (128, ST * W)).astype(BF16)], axis=1)
        rec_b = rec_all[k * GPC : (k + 1) * GPC]
        cb32_k = np.concatenate([
            np.broadcast_to(cb32_shared[None, :], (128, len(cb32_shared))),
            rec_b.reshape(128, 1),
        ], axis=1).astype(np.float32)
        in_maps.append(dict(xcc=xcc_p, cb16=np.ascontiguousarray(cb16_k),
                            cb32=np.ascontiguousarray(cb32_k)))

    nc = _build_program(S, C, R)

    kw = {}
    if _profile is not None:
        kw = dict(trace=True, **_profile)
    res = run_bass_kernel_spmd(nc, in_maps, list(range(N_CORES)), **kw)

    out = np.concatenate([res.results[k]["out"] for k in range(N_CORES)], axis=0)
    if _profile is not None:
        return out.astype(np.float32), res
    return out.astype(np.float32)


# revision 17
# speedup vs baseline: 2.5303x; 1.0422x over previous
"""Trainium2 Bass kernel for nn_ClockAwareGNN (segment_reduce).

Model (reference, fp32):
    gp   = segment_mean(x, batch) @ W_base + b_base            # [B, 1]
    h    = relu(clock @ W1 + b1) @ W2 + b2                     # [N, 16]
    cp   = segment_mean(h, batch)                              # [B, 16]
    out  = relu([gp | cp] @ W3 + b3) @ W4 + b4                 # [B, 1]

Everything after the segment reductions is affine in per-graph quantities, so
the heavy per-node work collapses to fused segment sums:
    Sx[g] = sum of x rows in graph g           (128 cols, fp8 payload)
    Sr[g] = sum of r rows in graph g           (R cols, fp8 hi + fp8 lo*512)
where r is either the raw clock (R=1; exact when b1 == 0 and clock >= 0 since
relu(c*W1) == c*relu(W1) elementwise for c >= 0) or the host-computed
relu(clock @ W1 + b1) (R=16 fallback). Graph node counts come from `batch` on
the host (they are index metadata), shipped as a per-graph 1/cnt constant.

Device strategy (per core, 8-way data-parallel by graph):
  - nodes arrive as 128-row tiles; batch ids are sorted so each 32-graph
    "window" owns a contiguous, per-window padded run of tiles.
  - the whole payload is fp8e4m3: x to ~2^-4 relative (segment-mean averages
    the quantization noise down by ~sqrt(n), n~2000) and clock as hi + lo*512
    pair; measured end-to-end rel err ~2.4e-3 vs the 2e-2 gate.
  - DVE builds one-hot assign tiles [128 nodes, 32 graphs] for a whole
    super-tile in one is_equal op (broadcast AP vs an iota pattern).
  - PE accumulates assign.T @ payload into PSUM [128 graphs, C] fp32 with ONE
    matmul per node-tile. Tiles are interleaved across the 4 windows so
    consecutive matmuls land in different PE column groups (tile_position)
    and overlap in the array.
  - tiny vector-engine epilogue computes the folded per-graph MLP.
"""

import math
import sys
import types

import numpy as np
import ml_dtypes

import concourse.bass as bass
import concourse.bacc as bacc
import concourse.tile as tile
from concourse import mybir
from concourse.bass_utils import run_bass_kernel_spmd


def _ensure_axon_hooks():
    """bass_utils' trace path does `from antenv.axon_hooks import ...`;
    some agent images lack that submodule. Install it (with the real NTFF
    hook when available) so trace=True degrades gracefully instead of
    raising ModuleNotFoundError."""
    try:
        import antenv  # noqa: F401
        import antenv.axon_hooks  # noqa: F401
        return
    except ImportError:
        pass
    try:
        import antenv
    except ImportError:
        return
    mod = types.ModuleType("antenv.axon_hooks")
    state = {"hook": None}
    mod.set_axon_ntff_profile_hook = lambda h: state.__setitem__("hook", h)
    mod.get_axon_ntff_profile_hook = lambda: state["hook"]
    sys.modules["antenv.axon_hooks"] = mod
    antenv.axon_hooks = mod
    try:
        from trn_agent_boot.trn_boot import _ntff_profile_via_ctypes
        mod.set_axon_ntff_profile_hook(
            _ntff_profile_via_ctypes("/opt/axon/libaxon_pjrt.so"))
    except Exception:
        pass
    # the trace path also uploads the NEFF dir to a bucket; in zero-egress
    # containers that raises — fall back to the local path.
    try:
        import concourse.bass_utils as _bu
        _orig_upload = _bu.upload_artifacts

        def _safe_upload(tmpdir):
            try:
                return _orig_upload(tmpdir)
            except Exception:
                return str(tmpdir)

        _bu.upload_artifacts = _safe_upload
    except Exception:
        pass


_ensure_axon_hooks()

BF16 = ml_dtypes.bfloat16
F8 = ml_dtypes.float8_e4m3

N_CORES = 8
N_GRAPHS = 1024
D = 128                 # feature dim of x
GPC = N_GRAPHS // N_CORES   # graphs per core = 128
W = 32                  # one-hot window width (PSUM partition alignment unit)
WPC = GPC // W          # windows per core = 4
ST = 64                 # node-tiles per DMA super-tile (ST % WPC == 0)
LO_SCALE = 512.0        # fp8 lo-correction pre-scale (2^9)


def _build_program(S, C, R):
    """Build the SPMD Bass/Tile program. Shapes are static; per-core data
    differences live entirely in the input tensors.

    S: number of super-tiles (each ST node-tiles of 128 nodes)
    C: fp8 payload column count = 128 + 2*R
    """
    fp32 = mybir.dt.float32
    bf16 = mybir.dt.bfloat16
    f8 = mybir.dt.float8e4
    n_tiles = S * ST

    nc = bacc.Bacc("TRN2", target_bir_lowering=False, debug=False,
                   num_devices=N_CORES)

    # bf16 block: [brall (S*ST) | iota (W)]; fp32 block:
    # [wb (D) | v1 (32) | m2 (R*32) | v0 (32) | w4 (32) | bb | b4 | rec]
    NB = S * ST + W
    NF = D + 32 + R * 32 + 32 + 32 + 3
    xcc = nc.dram_tensor("xcc", [S, 128, ST * C], f8, kind="ExternalInput").ap()
    cb16 = nc.dram_tensor("cb16", [128, NB], bf16, kind="ExternalInput").ap()
    cb32 = nc.dram_tensor("cb32", [128, NF], fp32, kind="ExternalInput").ap()
    # out rides as 4 rows of 32 (from the 32x32 block transpose of the
    # per-graph column) so the final store is 4 single-partition descriptors
    # instead of a 128-partition spray with 16 straggling HBM write receipts
    out_d = nc.dram_tensor("out", [4, 32], fp32, kind="ExternalOutput").ap()

    with tile.TileContext(nc) as tc:
        with (
            tc.tile_pool(name="consts", bufs=1) as cpool,
            tc.tile_pool(name="xin", bufs=6) as xpool,
            tc.tile_pool(name="assign", bufs=1) as apool,
            tc.tile_pool(name="epi", bufs=1) as epool,
            tc.tile_pool(name="ps", bufs=1, space="PSUM") as ppool,
        ):
            # ---- constants: two batched DMAs instead of eleven small ones ----
            cb16_t = cpool.tile([128, NB], bf16, tag="cb16")
            nc.sync.dma_start(cb16_t[:], cb16)
            cb32_t = cpool.tile([128, NF], fp32, tag="cb32")
            nc.scalar.dma_start(cb32_t[:], cb32)
            brall = cb16_t[:, 0 : S * ST]
            iota_t = cb16_t[:, S * ST : S * ST + W]
            o = 0
            wb_t = cb32_t[:, o : o + D]; o += D
            v1_t = cb32_t[:, o : o + 32]; o += 32
            m2_t = cb32_t[:, o : o + R * 32]; o += R * 32
            v0_t = cb32_t[:, o : o + 32]; o += 32
            w4_t = cb32_t[:, o : o + 32]; o += 32
            bbt = cb32_t[:, o : o + 1]; o += 1
            b4t = cb32_t[:, o : o + 1]; o += 1
            rect = cb32_t[:, o : o + 1]; o += 1

            psum = ppool.tile([128, C], fp32, tag="acc")

            # init matmul: zero weights x zero rhs, start=True claims the
            # whole bank's has_written bits so all later matmuls (start=False)
            # overwrite-on-first-touch / accumulate-after, independent of
            # window interleaving.
            zw = cpool.tile([128, 128], bf16, tag="zw")
            nc.vector.memset(zw[:], 0.0)
            zr = cpool.tile([128, C], bf16, tag="zr")
            nc.vector.memset(zr[:], 0.0)
            nc.tensor.matmul(psum[:, :], zw[:], zr[:], start=True, stop=False)

            # ---- one-hot assign tiles, built up-front off the critical path:
            # they depend only on constants, so the DVE runs ahead of the
            # DMA/PE pipeline instead of pacing it per super-tile.
            # asg[p, t, j] = (iota[j] == br[p, s*ST + t])
            asgs = []
            for s in range(S):
                asg = apool.tile([128, ST * W], bf16, tag=f"asg{s}")
                nc.vector.tensor_tensor(
                    asg[:].rearrange("p (t j) -> p t j", j=W),
                    iota_t.rearrange("p (o j) -> p o j", o=1)
                        .to_broadcast((128, ST, W)),
                    brall[:, s * ST : (s + 1) * ST]
                        .rearrange("p (t o) -> p t o", o=1)
                        .to_broadcast((128, ST, W)),
                    op=mybir.AluOpType.is_equal,
                )
                asgs.append(asg)

            # ---- main loop ----
            H = (ST // 2) * C
            for s in range(S):
                xt = xpool.tile([128, ST * C], f8, tag="xt")
                # split each super across BOTH HWDGE rings (SP + ACT) so the
                # two rings stream one super concurrently; subtile deps let
                # the first half's matmuls start before the second half lands
                nc.sync.dma_start(xt[:, 0:H], xcc[s][:, 0:H])
                nc.scalar.dma_start(xt[:, H : 2 * H], xcc[s][:, H : 2 * H])
                asg = asgs[s]
                for t in range(ST):
                    i = s * ST + t
                    w = i % WPC         # column-group interleave across windows
                    last = i == n_tiles - 1
                    nc.tensor.matmul(
                        psum[w * W : (w + 1) * W, 0:C],
                        asg[:, t * W : (t + 1) * W],
                        xt[:, t * C : (t + 1) * C],
                        start=False,
                        stop=last,
                        tile_position=(0, w * W),
                    )

            # ---- epilogue (per-graph folded MLP) ----
            sb = epool.tile([128, C], fp32, tag="sb")
            nc.vector.tensor_copy(sb[:], psum[:])

            # Sr = hi_sums + lo_sums / LO_SCALE
            slo = epool.tile([128, R], fp32, tag="slo")
            nc.vector.tensor_scalar_mul(slo[:], sb[:, D + R : D + 2 * R], 1.0 / LO_SCALE)
            sr = epool.tile([128, R], fp32, tag="sr")
            nc.vector.tensor_add(sr[:], sb[:, D : D + R], slo[:])

            mx = epool.tile([128, D], fp32, tag="mx")
            nc.vector.tensor_scalar_mul(mx[:], sb[:, 0:D], rect)
            mr = epool.tile([128, R], fp32, tag="mr")
            nc.vector.tensor_scalar_mul(mr[:], sr[:], rect)

            # gp = rowsum(mean_x * W_base) + b_base
            t1 = epool.tile([128, D], fp32, tag="t1")
            nc.vector.tensor_mul(t1[:], mx[:], wb_t)
            gp = epool.tile([128, 1], fp32, tag="gp")
            nc.vector.tensor_reduce(gp[:], t1[:], axis=mybir.AxisListType.X,
                                    op=mybir.AluOpType.add)
            nc.vector.tensor_add(gp[:], gp[:], bbt)

            # pre = gp*v1 + sum_j mr[:,j]*M2[j] + v0
            pre = epool.tile([128, 32], fp32, tag="pre")
            nc.vector.tensor_scalar_mul(pre[:], v1_t, gp[:])
            tmp = epool.tile([128, 32], fp32, tag="tmp")
            for j in range(R):
                nc.vector.tensor_scalar(
                    tmp[:], m2_t[:, j * 32 : (j + 1) * 32], mr[:, j : j + 1], None,
                    op0=mybir.AluOpType.mult,
                )
                nc.vector.tensor_add(pre[:], pre[:], tmp[:])
            nc.vector.tensor_add(pre[:], pre[:], v0_t)

            act = epool.tile([128, 32], fp32, tag="act")
            nc.scalar.activation(act[:], pre[:], mybir.ActivationFunctionType.Relu)

            # out = rowsum(act * W4) + b4
            nc.vector.tensor_mul(act[:], act[:], w4_t)
            oo = epool.tile([128, 32], fp32, tag="oo")
            nc.vector.memset(oo[:], 0.0)
            nc.vector.tensor_reduce(oo[:, 0:1], act[:], axis=mybir.AxisListType.X,
                                    op=mybir.AluOpType.add)
            nc.vector.tensor_add(oo[:, 0:1], oo[:, 0:1], b4t)

            # 32x32 block transpose: row 32*a of oot holds graphs 32a..32a+31,
            # so the store is 4 contiguous single-partition rows
            oot = epool.tile([128, 32], fp32, tag="oot")
            nc.vector.transpose(oot[:], oo[:])
            for a in range(4):
                nc.sync.dma_start(out_d[a : a + 1, :], oot[32 * a : 32 * a + 1, :])

    nc.compile()
    return nc


def kernel(x, clock_period, batch, W_base, b_base, W1, b1, W2, b2, W3, b3, W4, b4,
           _profile=None):
    x = np.asarray(x, np.float32)
    clock = np.asarray(clock_period, np.float32).reshape(-1)
    batch = np.asarray(batch, np.int32)
    W_base = np.asarray(W_base, np.float32)
    W1 = np.asarray(W1, np.float32); b1 = np.asarray(b1, np.float32)
    W2 = np.asarray(W2, np.float32); b2 = np.asarray(b2, np.float32)
    W3 = np.asarray(W3, np.float32); b3 = np.asarray(b3, np.float32)
    W4 = np.asarray(W4, np.float32); b4 = np.asarray(b4, np.float32)
    hid = W1.shape[1]

    # r-path: exact algebraic fold when relu(c*W1 + b1) == c * relu(W1)
    fold = bool(np.all(b1 == 0.0)) and bool(clock.min() >= 0.0)
    if fold:
        R = 1
        r32 = clock[:, None]                                   # [N, 1]
        q = np.maximum(W1, 0.0) @ W2                           # [1, hid]
        M2 = q @ W3[1:, :]                                     # [1, 32]
        v0 = b2 @ W3[1:, :] + b3                               # [32]
    else:
        R = hid
        r32 = np.maximum(clock[:, None] @ W1 + b1, 0.0)        # [N, hid]
        M2 = W2 @ W3[1:, :]                                    # [hid, 32]
        v0 = b2 @ W3[1:, :] + b3

    C = D + 2 * R           # [x | r_hi | r_lo], all fp8e4m3

    # ---- shard by graph; window padding so tile->window map is static ----
    cut = np.searchsorted(batch, np.arange(0, N_GRAPHS + 1, W))
    T_w = int(math.ceil(np.diff(cut).max() / 128.0))
    tpw = ST // WPC         # tiles of one window inside one super-tile
    while T_w % tpw:
        T_w += 1
    n_tiles = WPC * T_w
    S = n_tiles // ST

    gcut = np.searchsorted(batch, np.arange(0, N_GRAPHS + 1))
    cnt = np.diff(gcut).astype(np.float32)
    rec_all = (1.0 / np.maximum(cnt, 1.0)).astype(np.float32)

    x8 = x.astype(F8)
    rhi = r32.astype(F8)
    rlo = ((r32 - rhi.astype(np.float32)) * LO_SCALE).astype(F8)

    in_maps = []
    # shared constant blocks
    iota_c = np.arange(W, dtype=np.float32)
    cb32_shared = np.concatenate([
        W_base[:, 0], W3[0, :], M2.reshape(-1), v0, W4[:, 0],
        [float(b_base.reshape(-1)[0])], [float(b4.reshape(-1)[0])],
    ]).astype(np.float32)

    for k in range(N_CORES):
        wx = np.zeros((WPC, T_w * 128, C), F8)
        wbr = np.full((WPC, T_w * 128), -1.0, BF16)
        for wi in range(WPC):
            gw = k * WPC + wi          # global window index
            s0, e0 = int(cut[gw]), int(cut[gw + 1])
            n = e0 - s0
            wx[wi, :n, 0:D] = x8[s0:e0]
            wx[wi, :n, D : D + R] = rhi[s0:e0]
            wx[wi, :n, D + R : D + 2 * R] = rlo[s0:e0]
            wbr[wi, :n] = (batch[s0:e0] - gw * W).astype(BF16)
        # window-interleaved tile order: tile i = s*ST + t belongs to window
        # i % WPC at within-window slot i // WPC; each SBUF partition line is
        # contiguous in DRAM.
        xcc_p = np.ascontiguousarray(
            wx.reshape(WPC, S, tpw, 128, C).transpose(1, 3, 2, 0, 4)
        ).reshape(S, 128, ST * C)
        brs_p = np.ascontiguousarray(
            wbr.reshape(WPC, S, tpw, 128).transpose(3, 1, 2, 0)
        ).reshape(128, S * ST)
        cb16_k = np.concatenate(
            [brs_p.astype(BF16),
             np.broadcast_to(iota_c[None, :], (128, W)).astype(BF16)], axis=1)
        rec_b = rec_all[k * GPC : (k + 1) * GPC]
        cb32_k = np.concatenate([
            np.broadcast_to(cb32_shared[None, :], (128, len(cb32_shared))),
            rec_b.reshape(128, 1),
        ], axis=1).astype(np.float32)
        in_maps.append(dict(xcc=xcc_p, cb16=np.ascontiguousarray(cb16_k),
                            cb32=np.ascontiguousarray(cb32_k)))

    nc = _build_program(S, C, R)

    kw = {}
    if _profile is not None:
        kw = dict(trace=True, **_profile)
    res = run_bass_kernel_spmd(nc, in_maps, list(range(N_CORES)), **kw)

    out = np.concatenate(
        [res.results[k]["out"].reshape(GPC, 1) for k in range(N_CORES)], axis=0)
    if _profile is not None:
        return out.astype(np.float32), res
    return out.astype(np.float32)


# revision 20
# speedup vs baseline: 2.5362x; 1.0024x over previous
"""Trainium2 Bass kernel for nn_ClockAwareGNN (segment_reduce).

Model (reference, fp32):
    gp   = segment_mean(x, batch) @ W_base + b_base            # [B, 1]
    h    = relu(clock @ W1 + b1) @ W2 + b2                     # [N, 16]
    cp   = segment_mean(h, batch)                              # [B, 16]
    out  = relu([gp | cp] @ W3 + b3) @ W4 + b4                 # [B, 1]

Everything after the segment reductions is affine in per-graph quantities, so
the heavy per-node work collapses to fused segment sums:
    Sx[g] = sum of x rows in graph g           (128 cols, fp8 payload)
    Sr[g] = sum of r rows in graph g           (R cols, fp8 hi + fp8 lo*512)
where r is either the raw clock (R=1; exact when b1 == 0 and clock >= 0 since
relu(c*W1) == c*relu(W1) elementwise for c >= 0) or the host-computed
relu(clock @ W1 + b1) (R=16 fallback). Graph node counts come from `batch` on
the host (they are index metadata), shipped as a per-graph 1/cnt constant.

Device strategy (per core, 8-way data-parallel by graph):
  - nodes arrive as 128-row tiles; batch ids are sorted so each 32-graph
    "window" owns a contiguous, per-window padded run of tiles.
  - the whole payload is fp8e4m3: x to ~2^-4 relative (segment-mean averages
    the quantization noise down by ~sqrt(n), n~2000) and clock as hi + lo*512
    pair; measured end-to-end rel err ~2.4e-3 vs the 2e-2 gate.
  - DVE builds one-hot assign tiles [128 nodes, 32 graphs] for a whole
    super-tile in one is_equal op (broadcast AP vs an iota pattern).
  - PE accumulates assign.T @ payload into PSUM [128 graphs, C] fp32 with ONE
    matmul per node-tile. Tiles are interleaved across the 4 windows so
    consecutive matmuls land in different PE column groups (tile_position)
    and overlap in the array.
  - tiny vector-engine epilogue computes the folded per-graph MLP.
"""

import math
import sys
import types

import numpy as np
import ml_dtypes

import concourse.bass as bass
import concourse.bacc as bacc
import concourse.tile as tile
from concourse import mybir
from concourse.bass_utils import run_bass_kernel_spmd


def _ensure_axon_hooks():
    """bass_utils' trace path does `from antenv.axon_hooks import ...`;
    some agent images lack that submodule. Install it (with the real NTFF
    hook when available) so trace=True degrades gracefully instead of
    raising ModuleNotFoundError."""
    try:
        import antenv  # noqa: F401
        import antenv.axon_hooks  # noqa: F401
        return
    except ImportError:
        pass
    try:
        import antenv
    except ImportError:
        return
    mod = types.ModuleType("antenv.axon_hooks")
    state = {"hook": None}
    mod.set_axon_ntff_profile_hook = lambda h: state.__setitem__("hook", h)
    mod.get_axon_ntff_profile_hook = lambda: state["hook"]
    sys.modules["antenv.axon_hooks"] = mod
    antenv.axon_hooks = mod
    try:
        from trn_agent_boot.trn_boot import _ntff_profile_via_ctypes
        mod.set_axon_ntff_profile_hook(
            _ntff_profile_via_ctypes("/opt/axon/libaxon_pjrt.so"))
    except Exception:
        pass
    # the trace path also uploads the NEFF dir to a bucket; in zero-egress
    # containers that raises — fall back to the local path.
    try:
        import concourse.bass_utils as _bu
        _orig_upload = _bu.upload_artifacts

        def _safe_upload(tmpdir):
            try:
                return _orig_upload(tmpdir)
            except Exception:
                return str(tmpdir)

        _bu.upload_artifacts = _safe_upload
    except Exception:
        pass


_ensure_axon_hooks()

BF16 = ml_dtypes.bfloat16
F8 = ml_dtypes.float8_e4m3

N_CORES = 8
N_GRAPHS = 1024
D = 128                 # feature dim of x
GPC = N_GRAPHS // N_CORES   # graphs per core = 128
W = 32                  # one-hot window width (PSUM partition alignment unit)
WPC = GPC // W          # windows per core = 4
ST = 64                 # node-tiles per DMA super-tile (ST % WPC == 0)
LO_SCALE = 512.0        # fp8 lo-correction pre-scale (2^9)


def _build_program(S, C, R):
    """Build the SPMD Bass/Tile program. Shapes are static; per-core data
    differences live entirely in the input tensors.

    S: number of super-tiles (each ST node-tiles of 128 nodes)
    C: fp8 payload column count = 128 + 2*R
    """
    fp32 = mybir.dt.float32
    bf16 = mybir.dt.bfloat16
    f8 = mybir.dt.float8e4
    n_tiles = S * ST

    nc = bacc.Bacc("TRN2", target_bir_lowering=False, debug=False,
                   num_devices=N_CORES)

    # bf16 block: [brall (S*ST) | iota (W)]; fp32 block:
    # [wb (D) | v1 (32) | m2 (R*32) | v0 (32) | w4 (32) | bb | b4 | rec]
    NB = S * ST + W
    NF = D + 32 + R * 32 + 32 + 32 + 3
    xcc = nc.dram_tensor("xcc", [S, 128, ST * C], f8, kind="ExternalInput").ap()
    cb16 = nc.dram_tensor("cb16", [128, NB], bf16, kind="ExternalInput").ap()
    cb32 = nc.dram_tensor("cb32", [128, NF], fp32, kind="ExternalInput").ap()
    # out rides as 4 rows of 32 (from the 32x32 block transpose of the
    # per-graph column) so the final store is 4 single-partition descriptors
    # instead of a 128-partition spray with 16 straggling HBM write receipts
    out_d = nc.dram_tensor("out", [4, 32], fp32, kind="ExternalOutput").ap()

    with tile.TileContext(nc) as tc:
        with (
            tc.tile_pool(name="consts", bufs=1) as cpool,
            tc.tile_pool(name="xin", bufs=8) as xpool,
            tc.tile_pool(name="assign", bufs=1) as apool,
            tc.tile_pool(name="epi", bufs=1) as epool,
            tc.tile_pool(name="ps", bufs=1, space="PSUM") as ppool,
        ):
            # ---- constants: two batched DMAs instead of eleven small ones,
            # both on the ACT ring so the x stream leads the SP ring ----
            cb32_t = cpool.tile([128, NF], fp32, tag="cb32")
            nc.scalar.dma_start(cb32_t[:], cb32)
            cb16_t = cpool.tile([128, NB], bf16, tag="cb16")
            nc.scalar.dma_start(cb16_t[:], cb16)
            brall = cb16_t[:, 0 : S * ST]
            iota_t = cb16_t[:, S * ST : S * ST + W]
            o = 0
            wb_t = cb32_t[:, o : o + D]; o += D
            v1_t = cb32_t[:, o : o + 32]; o += 32
            m2_t = cb32_t[:, o : o + R * 32]; o += R * 32
            v0_t = cb32_t[:, o : o + 32]; o += 32
            w4_t = cb32_t[:, o : o + 32]; o += 32
            bbt = cb32_t[:, o : o + 1]; o += 1
            b4t = cb32_t[:, o : o + 1]; o += 1
            rect = cb32_t[:, o : o + 1]; o += 1

            psum = ppool.tile([128, C], fp32, tag="acc")

            # init matmul: zero weights x zero rhs, start=True claims the
            # whole bank's has_written bits so all later matmuls (start=False)
            # overwrite-on-first-touch / accumulate-after, independent of
            # window interleaving.
            zw = cpool.tile([128, 128], bf16, tag="zw")
            nc.vector.memset(zw[:], 0.0)
            zr = cpool.tile([128, C], bf16, tag="zr")
            nc.vector.memset(zr[:], 0.0)
            nc.tensor.matmul(psum[:, :], zw[:], zr[:], start=True, stop=False)

            # ---- one-hot assign tiles, built up-front off the critical path:
            # they depend only on constants, so the DVE runs ahead of the
            # DMA/PE pipeline instead of pacing it per super-tile.
            # asg[p, t, j] = (iota[j] == br[p, s*ST + t])
            asgs = []
            for s in range(S):
                asg = apool.tile([128, ST * W], bf16, tag=f"asg{s}")
                nc.vector.tensor_tensor(
                    asg[:].rearrange("p (t j) -> p t j", j=W),
                    iota_t.rearrange("p (o j) -> p o j", o=1)
                        .to_broadcast((128, ST, W)),
                    brall[:, s * ST : (s + 1) * ST]
                        .rearrange("p (t o) -> p t o", o=1)
                        .to_broadcast((128, ST, W)),
                    op=mybir.AluOpType.is_equal,
                )
                asgs.append(asg)

            # ---- main loop ----
            H = (ST // 2) * C
            for s in range(S):
                xt = xpool.tile([128, ST * C], f8, tag="xt")
                # split each super across BOTH HWDGE rings (SP + ACT) so the
                # two rings stream one super concurrently; subtile deps let
                # the first half's matmuls start before the second half lands
                nc.sync.dma_start(xt[:, 0:H], xcc[s][:, 0:H])
                nc.scalar.dma_start(xt[:, H : 2 * H], xcc[s][:, H : 2 * H])
                asg = asgs[s]
                for t in range(ST):
                    i = s * ST + t
                    w = i % WPC         # column-group interleave across windows
                    last = i == n_tiles - 1
                    nc.tensor.matmul(
                        psum[w * W : (w + 1) * W, 0:C],
                        asg[:, t * W : (t + 1) * W],
                        xt[:, t * C : (t + 1) * C],
                        start=False,
                        stop=last,
                        tile_position=(0, w * W),
                    )

            # ---- epilogue (per-graph folded MLP); reads PSUM directly ----
            # Sr = hi_sums + lo_sums / LO_SCALE, then mean via rec
            slo = epool.tile([128, R], fp32, tag="slo")
            nc.vector.tensor_scalar_mul(slo[:], psum[:, D + R : D + 2 * R],
                                        1.0 / LO_SCALE)
            sr = epool.tile([128, R], fp32, tag="sr")
            nc.vector.tensor_add(sr[:], psum[:, D : D + R], slo[:])
            mr = epool.tile([128, R], fp32, tag="mr")
            nc.vector.tensor_scalar_mul(mr[:], sr[:], rect)

            # gp = rowsum(Sx * W_base) * rec + b_base
            t1 = epool.tile([128, D], fp32, tag="t1")
            nc.vector.tensor_mul(t1[:], psum[:, 0:D], wb_t)
            gp = epool.tile([128, 1], fp32, tag="gp")
            nc.vector.tensor_reduce(gp[:], t1[:], axis=mybir.AxisListType.X,
                                    op=mybir.AluOpType.add)
            nc.vector.tensor_scalar(gp[:], gp[:], rect, bbt,
                                    op0=mybir.AluOpType.mult,
                                    op1=mybir.AluOpType.add)

            # pre = gp*v1 + sum_j mr[:,j]*M2[j] + v0
            pre = epool.tile([128, 32], fp32, tag="pre")
            nc.vector.tensor_scalar_mul(pre[:], v1_t, gp[:])
            tmp = epool.tile([128, 32], fp32, tag="tmp")
            for j in range(R):
                nc.vector.tensor_scalar(
                    tmp[:], m2_t[:, j * 32 : (j + 1) * 32], mr[:, j : j + 1], None,
                    op0=mybir.AluOpType.mult,
                )
                nc.vector.tensor_add(pre[:], pre[:], tmp[:])
            nc.vector.tensor_add(pre[:], pre[:], v0_t)

            act = epool.tile([128, 32], fp32, tag="act")
            nc.scalar.activation(act[:], pre[:], mybir.ActivationFunctionType.Relu)

            # out = rowsum(act * W4) + b4
            nc.vector.tensor_mul(act[:], act[:], w4_t)
            oo = epool.tile([128, 32], fp32, tag="oo")
            nc.vector.memset(oo[:], 0.0)
            nc.vector.tensor_reduce(oo[:, 0:1], act[:], axis=mybir.AxisListType.X,
                                    op=mybir.AluOpType.add)
            nc.vector.tensor_add(oo[:, 0:1], oo[:, 0:1], b4t)

            # 32x32 block transpose: row 32*a of oot holds graphs 32a..32a+31,
            # so the store is 4 contiguous single-partition rows (2 per ring)
            oot = epool.tile([128, 32], fp32, tag="oot")
            nc.vector.transpose(oot[:], oo[:])
            for a in range(4):
                eng = nc.sync if a % 2 == 0 else nc.scalar
                eng.dma_start(out_d[a : a + 1, :], oot[32 * a : 32 * a + 1, :])

    nc.compile()
    return nc


def kernel(x, clock_period, batch, W_base, b_base, W1, b1, W2, b2, W3, b3, W4, b4,
           _profile=None):
    x = np.asarray(x, np.float32)
    clock = np.asarray(clock_period, np.float32).reshape(-1)
    batch = np.asarray(batch, np.int32)
    W_base = np.asarray(W_base, np.float32)
    W1 = np.asarray(W1, np.float32); b1 = np.asarray(b1, np.float32)
    W2 = np.asarray(W2, np.float32); b2 = np.asarray(b2, np.float32)
    W3 = np.asarray(W3, np.float32); b3 = np.asarray(b3, np.float32)
    W4 = np.asarray(W4, np.float32); b4 = np.asarray(b4, np.float32)
    hid = W1.shape[1]

    # r-path: exact algebraic fold when relu(c*W1 + b1) == c * relu(W1)
    fold = bool(np.all(b1 == 0.0)) and bool(clock.min() >= 0.0)
    if fold:
        R = 1
        r32 = clock[:, None]                                   # [N, 1]
        q = np.maximum(W1, 0.0) @ W2                           # [1, hid]
        M2 = q @ W3[1:, :]                                     # [1, 32]
        v0 = b2 @ W3[1:, :] + b3                               # [32]
    else:
        R = hid
        r32 = np.maximum(clock[:, None] @ W1 + b1, 0.0)        # [N, hid]
        M2 = W2 @ W3[1:, :]                                    # [hid, 32]
        v0 = b2 @ W3[1:, :] + b3

    C = D + 2 * R           # [x | r_hi | r_lo], all fp8e4m3

    # ---- shard by graph; window padding so tile->window map is static ----
    cut = np.searchsorted(batch, np.arange(0, N_GRAPHS + 1, W))
    T_w = int(math.ceil(np.diff(cut).max() / 128.0))
    tpw = ST // WPC         # tiles of one window inside one super-tile
    while T_w % tpw:
        T_w += 1
    n_tiles = WPC * T_w
    S = n_tiles // ST

    gcut = np.searchsorted(batch, np.arange(0, N_GRAPHS + 1))
    cnt = np.diff(gcut).astype(np.float32)
    rec_all = (1.0 / np.maximum(cnt, 1.0)).astype(np.float32)

    x8 = x.astype(F8)
    rhi = r32.astype(F8)
    rlo = ((r32 - rhi.astype(np.float32)) * LO_SCALE).astype(F8)

    in_maps = []
    # shared constant blocks
    iota_c = np.arange(W, dtype=np.float32)
    cb32_shared = np.concatenate([
        W_base[:, 0], W3[0, :], M2.reshape(-1), v0, W4[:, 0],
        [float(b_base.reshape(-1)[0])], [float(b4.reshape(-1)[0])],
    ]).astype(np.float32)

    for k in range(N_CORES):
        wx = np.zeros((WPC, T_w * 128, C), F8)
        wbr = np.full((WPC, T_w * 128), -1.0, BF16)
        for wi in range(WPC):
            gw = k * WPC + wi          # global window index
            s0, e0 = int(cut[gw]), int(cut[gw + 1])
            n = e0 - s0
            wx[wi, :n, 0:D] = x8[s0:e0]
            wx[wi, :n, D : D + R] = rhi[s0:e0]
            wx[wi, :n, D + R : D + 2 * R] = rlo[s0:e0]
            wbr[wi, :n] = (batch[s0:e0] - gw * W).astype(BF16)
        # window-interleaved tile order: tile i = s*ST + t belongs to window
        # i % WPC at within-window slot i // WPC; each SBUF partition line is
        # contiguous in DRAM.
        xcc_p = np.ascontiguousarray(
            wx.reshape(WPC, S, tpw, 128, C).transpose(1, 3, 2, 0, 4)
        ).reshape(S, 128, ST * C)
        brs_p = np.ascontiguousarray(
            wbr.reshape(WPC, S, tpw, 128).transpose(3, 1, 2, 0)
        ).reshape(128, S * ST)
        cb16_k = np.concatenate(
            [brs_p.astype(BF16),
             np.broadcast_to(iota_c[None, :], (128, W)).astype(BF16)], axis=1)
        rec_b = rec_all[k * GPC : (k + 1) * GPC]
        cb32_k = np.concatenate([
            np.broadcast_to(cb32_shared[None, :], (128, len(cb32_shared))),
            rec_b.reshape(128, 1),
        ], axis=1).astype(np.float32)
        in_maps.append(dict(xcc=xcc_p, cb16=np.ascontiguousarray(cb16_k),
                            cb32=np.ascontiguousarray(cb32_k)))

    nc = _build_program(S, C, R)

    kw = {}
    if _profile is not None:
        kw = dict(trace=True, **_profile)
    res = run_bass_kernel_spmd(nc, in_maps, list(range(N_CORES)), **kw)

    out = np.concatenate(
        [res.results[k]["out"].reshape(GPC, 1) for k in range(N_CORES)], axis=0)
    if _profile is not None:
        return out.astype(np.float32), res
    return out.astype(np.float32)
